# revision 3
# baseline (speedup 1.0000x reference)
"""Trainium2 Bass kernel for the AOI (attention-over-interactions) module.

Strategy (8 NeuronCores, data-parallel over question groups):
  - B=128 options = 32 self-contained groups of 4; 4 "group slots" x 8 cores.
  - Host assigns groups to slots to minimize per-slot ragged length maxima
    (same SPMD graph on all cores; per-(slot,option) key lengths baked into
    the instruction stream as max over the 8 cores at that slot).
  - Per-core masked-softmax semantics are matched to the reference exactly
    (max over masked-zeros, i.e. M = max(0, row max), and the +1e-13
    renormalization epsilon scaled by the full-softmax partition function)
    via a per-core mask row folded into the attention matmul chain plus one
    extra "epsilon column".
  - Mixed precision: bf16 for the attention-score path; fp8e4m3 DoubleRow
    matmuls (2 K-tiles per instruction, 2x PE rate) for the Wd3 stage and
    the final W1/W2 fuse stage, with power-of-two scaling folded into the
    activation scale/bias parameters.
"""

import math
import sys

for _p in ("/opt/trn_rl_repo", "/opt/pypackages"):
    if _p not in sys.path:
        sys.path.append(_p)

import numpy as np
import ml_dtypes

B, S, H = 128, 256, 768
N_CORES = 8
GPC = 4           # group slots per core
HC = H // 128     # 6 h-chunks
BF16 = ml_dtypes.bfloat16
FP8 = ml_dtypes.float8_e4m3   # TRN float8e4; max 240, overflow -> inf

SCW = 2048.0      # weight scale for wd3/w1 (fp8)
SCW2 = 128.0      # weight scale for w2 (fp8)
SCP = 16.0        # activation scale for p (fp8)
SCK = 64.0        # activation scale for ock (fp8)
# oc psum scale = SCK*SCW = 2^17 ; final psum scale = 1*SCW = SCP*SCW2 = 2^11

_GRAPH_CACHE = {}


def _clip8(x, scale):
    return np.clip(np.asarray(x, np.float32) * scale, -224.0, 224.0).astype(FP8)


def _assign_groups(glens):
    """Partition 32 groups into 4 slots of 8 minimizing sum_slot sum_o max_core len.

    Returns slots: list[4] of list[8] group ids (core c takes slots[g][c]).
    """
    rng = np.random.default_rng(0)
    n_groups = glens.shape[0]

    def cost(assign):
        c = 0
        for g in range(GPC):
            ids = assign[g]
            c += int(glens[ids].max(axis=0).sum())
        return c

    best, best_cost = None, None
    for trial in range(6):
        if trial == 0:
            order = np.argsort(-glens.sum(axis=1))
        else:
            order = rng.permutation(n_groups)
        assign = [list(order[g * 8:(g + 1) * 8]) for g in range(GPC)]
        # 2-swap hill climbing across slots
        improved = True
        cur = cost(assign)
        it = 0
        while improved and it < 60:
            improved = False
            it += 1
            for ga in range(GPC):
                for gb in range(ga + 1, GPC):
                    for ia in range(8):
                        for ib in range(8):
                            assign[ga][ia], assign[gb][ib] = assign[gb][ib], assign[ga][ia]
                            nc_ = cost(assign)
                            if nc_ < cur:
                                cur = nc_
                                improved = True
                            else:
                                assign[ga][ia], assign[gb][ib] = assign[gb][ib], assign[ga][ia]
        if best_cost is None or cur < best_cost:
            best_cost, best = cur, [list(a) for a in assign]
    return best


def _build_graph(slot_lens):
    """Build + compile the SPMD Bacc graph. slot_lens: [GPC][4] ints (1..256)."""
    import concourse.bass as bass
    import concourse.bacc as bacc
    import concourse.mybir as mybir
    import concourse.tile as tile
    from concourse.masks import make_identity

    f32 = mybir.dt.float32
    fr = mybir.dt.float32r
    bf = mybir.dt.bfloat16
    f8 = mybir.dt.float8e4
    AX = mybir.AxisListType
    AF = mybir.ActivationFunctionType
    DR = mybir.MatmulPerfMode.DoubleRow

    nc = bacc.Bacc("TRN2", target_bir_lowering=False, debug=False,
                   num_devices=N_CORES)

    p_bf_d = nc.dram_tensor("p_bf", [GPC, HC, 128, 4, S], bf, kind="ExternalInput")
    p_f8_d = nc.dram_tensor("p_f8", [GPC, HC, 128, 4, S], f8, kind="ExternalInput")
    mask_d = nc.dram_tensor("maskrow", [GPC, 1, 4, S + 1], bf, kind="ExternalInput")
    wt_d = nc.dram_tensor("wt", [128, HC, HC, 128], bf, kind="ExternalInput")
    wd_d = nc.dram_tensor("wd", [128, HC, H], bf, kind="ExternalInput")
    wd3_d = nc.dram_tensor("wd3", [3, 128, 3, HC, 2, 128], f8, kind="ExternalInput")
    w1_d = nc.dram_tensor("w1", [128, 3, HC, 2, 128], f8, kind="ExternalInput")
    w2_d = nc.dram_tensor("w2", [128, 3, HC, 2, 128], f8, kind="ExternalInput")
    bias_d = nc.dram_tensor("biases", [128, 4, HC], f32, kind="ExternalInput")
    out_d = nc.dram_tensor("out", [GPC, 4, HC, 128, S], f32, kind="ExternalOutput")

    with tile.TileContext(nc) as tc:
        with (
            tc.tile_pool(name="const", bufs=1) as constp,
            tc.tile_pool(name="wres", bufs=1) as wres,
            tc.tile_pool(name="wstream", bufs=2) as wstream,
            tc.tile_pool(name="gin", bufs=1) as gin,
            tc.tile_pool(name="act", bufs=1) as act,
            tc.tile_pool(name="smp", bufs=2) as smp,
            tc.tile_pool(name="fin", bufs=2) as fin,
            tc.tile_pool(name="patt", bufs=4, space="PSUM") as patt,
            tc.tile_pool(name="psbig", bufs=1, space="PSUM") as psbig,
            tc.tile_pool(name="pshalf", bufs=1, space="PSUM") as pshalf,
            tc.tile_pool(name="pstr", bufs=1, space="PSUM") as pstr,
        ):
            ident_f = constp.tile([128, 128], f32)
            make_identity(nc, ident_f[:])
            ident = constp.tile([128, 128], fr)
            nc.vector.tensor_copy(ident[:], ident_f[:])
            ones_bf = constp.tile([1, 128], bf)
            nc.vector.memset(ones_bf[:], 1.0)
            biases = constp.tile([128, 4, HC], f32)
            nc.sync.dma_start(biases[:], bias_d.ap())
            bt_b = biases[:, 0, :]
            bd_b = biases[:, 1, :]   # pre-scaled by SCK host-side
            bd3_b = biases[:, 2, :]
            b12_b = biases[:, 3, :]

            wt_sb = wres.tile([128, HC, HC, 128], bf)
            nc.sync.dma_start(wt_sb[:], wt_d.ap())

            # first group's inputs land before the remaining weights so the
            # tensor engine can start at ~8us instead of ~24us
            pbf_t, pf8_t, msk_t = [], [], []
            for g in range(GPC):
                pbf_t.append(gin.tile([128, HC, 4, S], bf, tag="pbf", bufs=2,
                                      name=f"pbf{g}"))
                pf8_t.append(gin.tile([128, HC, 4, S], f8, tag="pf8", bufs=2,
                                      name=f"pf8{g}"))
                msk_t.append(gin.tile([1, 4, S + 1], bf, tag="msk", bufs=2,
                                      name=f"msk{g}"))

            def load_group(g):
                for hc in range(HC):
                    nc.sync.dma_start(pbf_t[g][:, hc, :, :], p_bf_d.ap()[g][hc])
                nc.sync.dma_start(msk_t[g][:], mask_d.ap()[g])
                for hc in range(HC):
                    nc.sync.dma_start(pf8_t[g][:, hc, :, :], p_f8_d.ap()[g][hc])

            load_group(0)

            wd_sb = wres.tile([128, HC, H], bf)
            nc.sync.dma_start(wd_sb[:], wd_d.ap())
            w1_sb = wres.tile([128, 3, HC, 2, 128], f8)
            nc.sync.dma_start(w1_sb[:], w1_d.ap())
            w2_sb = wres.tile([128, 3, HC, 2, 128], f8)
            nc.sync.dma_start(w2_sb[:], w2_d.ap())

            for g in range(GPC):
                L = [int(x) for x in slot_lens[g]]
                tcs = [max(1, math.ceil(l / 128)) for l in L]

                pbf, pf8, msk = pbf_t[g], pf8_t[g], msk_t[g]
                if g + 1 < GPC:
                    load_group(g + 1)

                # ---- trans_t (feature-major, bf16): ttT[h', t] = Wt @ pT + bt
                ttT = act.tile([128, HC, 4, S], bf, tag="ttT", bufs=2)
                for m in range(HC):
                    for o in range(4):
                        ps = patt.tile([128, 257], f32, tag="patt")
                        for hc in range(HC):
                            nc.tensor.matmul(
                                ps[:, 0:L[o]],
                                wt_sb[:, hc, m, :],
                                pbf[:, hc, o, 0:L[o]],
                                start=(hc == 0), stop=(hc == HC - 1),
                            )
                        nc.scalar.activation(
                            ttT[:, m, o, 0:L[o]], ps[:, 0:L[o]],
                            AF.Identity, bias=bt_b[:, m:m + 1],
                        )

                # ---- trans_d (natural [t, h], bf16): td = pT^T @ WdT
                td = act.tile([128, 4, 2, H], bf, tag="td")
                for o in range(4):
                    for tcx in range(tcs[o]):
                        w = min(128, L[o] - tcx * 128)
                        ps = psbig.tile([128, 1024], f32, tag="psbig")
                        for hc in range(HC):
                            lhs = pbf[:, hc, o, tcx * 128: tcx * 128 + w]
                            nc.tensor.matmul(ps[0:w, 0:512], lhs, wd_sb[:, hc, 0:512],
                                             start=(hc == 0), stop=(hc == HC - 1))
                            nc.tensor.matmul(ps[0:w, 512:768], lhs, wd_sb[:, hc, 512:768],
                                             start=(hc == 0), stop=(hc == HC - 1))
                        nc.scalar.activation(td[0:w, o, tcx, :], ps[0:w, 0:768], AF.Copy)

                # stream Wd3 for this group
                wd3_sb = []
                for k in range(3):
                    t = wstream.tile([128, 3, HC, 2, 128], f8, tag=f"wd3_{k}", bufs=1)
                    nc.sync.dma_start(t[:], wd3_d.ap()[k])
                    wd3_sb.append(t)

                ocT = act.tile([128, HC, 4, S], f8, tag="ocT")
                for half in range(2):
                    ock = act.tile([128, 3, HC, 2, S], f8, tag="ock", bufs=2)
                    for io in range(2):
                        i = half * 2 + io
                        jlist = [j for j in range(4) if j != i]
                        # ---- att scores for the 3 partners, [s_m, t] layout
                        for jr, j in enumerate(jlist):
                            lj = L[j]
                            smc = []
                            for m in range(2):
                                ps = patt.tile([128, 257], f32, tag="patt")
                                nc.tensor.matmul(
                                    ps[:, 0:lj + 1],
                                    ones_bf[0:1, :], msk[0:1, j, 0:lj + 1],
                                    start=True, stop=False,
                                )
                                for hc in range(HC):
                                    nc.tensor.matmul(
                                        ps[:, 0:lj],
                                        pbf[:, hc, i, m * 128:(m + 1) * 128],
                                        ttT[:, hc, j, 0:lj],
                                        start=False, stop=(hc == HC - 1),
                                    )
                                stats = smp.tile([128, 16], f32, tag="stats", bufs=4)
                                nc.vector.tensor_reduce(
                                    stats[:, 0:1], ps[:, 0:lj], AX.X,
                                    mybir.AluOpType.max, negate=True)
                                # -M = min(0, -max)
                                nc.vector.tensor_scalar_min(stats[:, 0:1], stats[:, 0:1], 0.0)
                                e = smp.tile([128, 257], f32, tag="e", bufs=2)
                                nc.scalar.activation(
                                    e[:, 0:lj + 1], ps[:, 0:lj + 1],
                                    AF.Exp, bias=stats[:, 0:1], scale=1.0,
                                    accum_out=stats[:, 1:2])
                                nc.vector.reciprocal(stats[:, 2:3], stats[:, 1:2])
                                sm = smp.tile([128, 256], fr, tag="sm", bufs=4)
                                nc.vector.tensor_scalar_mul(sm[:, 0:lj], e[:, 0:lj],
                                                            stats[:, 2:3])
                                smc.append(sm)
                            smT = smp.tile([128, 2, 256], bf, tag="smT", bufs=2)
                            for tcx in range(tcs[j]):
                                w = min(128, lj - tcx * 128)
                                tp = pstr.tile([128, 256], fr, tag="pstr")
                                for m in range(2):
                                    nc.tensor.matmul(
                                        tp[0:w, m * 128:(m + 1) * 128],
                                        smc[m][:, tcx * 128: tcx * 128 + w],
                                        ident[:], is_transpose=True,
                                        start=(m == 0), stop=(m == 1))
                                nc.scalar.activation(smT[0:w, tcx, :], tp[0:w, :],
                                                     AF.Copy)
                            # ---- av: avT[h', s] += td_j^T(blocks) @ smT
                            for hc in range(HC):
                                aps = patt.tile([128, 257], f32, tag="patt")
                                for tcx in range(tcs[j]):
                                    w = min(128, lj - tcx * 128)
                                    nc.tensor.matmul(
                                        aps[:, 0:256],
                                        td[0:w, j, tcx, hc * 128:(hc + 1) * 128],
                                        smT[0:w, tcx, :],
                                        start=(tcx == 0), stop=(tcx == tcs[j] - 1))
                                nc.scalar.activation(
                                    ock[:, jr, hc, io, :], aps[:, 0:256],
                                    AF.Relu, bias=bd_b[:, hc:hc + 1], scale=SCK)

                    # ---- oc for this half (2 options), fp8 DoubleRow:
                    # ocT = (sum_k Wd3_k @ ock_k) * 2^-17 + bd3
                    for m in range(HC):
                        ops = pshalf.tile([128, 512], f32, tag="pshalf")
                        step = 0
                        for k in range(3):
                            for hp in range(3):
                                nc.tensor.matmul(
                                    ops[:, :], wd3_sb[k][:, hp, m, :, :],
                                    ock[:, k, 2 * hp:2 * hp + 2, :, :],
                                    start=(step == 0), stop=(step == 8),
                                    perf_mode=DR)
                                step += 1
                        for oo in range(2):
                            o = half * 2 + oo
                            nc.scalar.activation(
                                ocT[:, m, o, :], ops[:, oo * 256:(oo + 1) * 256],
                                AF.Identity, bias=bd3_b[:, m:m + 1],
                                scale=1.0 / (SCK * SCW))

                # ---- mid + final output, per m-chunk (fp8 DoubleRow matmuls)
                for m in range(HC):
                    zps = psbig.tile([128, 1024], f32, tag="psbig")
                    for hp in range(3):
                        nc.tensor.matmul(zps[:, 0:512], w1_sb[:, hp, m, :, :],
                                         ocT[:, 2 * hp:2 * hp + 2, 0:2, :],
                                         start=(hp == 0), stop=False, perf_mode=DR)
                        nc.tensor.matmul(zps[:, 512:1024], w1_sb[:, hp, m, :, :],
                                         ocT[:, 2 * hp:2 * hp + 2, 2:4, :],
                                         start=(hp == 0), stop=False, perf_mode=DR)
                    for hp in range(3):
                        nc.tensor.matmul(zps[:, 0:512], w2_sb[:, hp, m, :, :],
                                         pf8[:, 2 * hp:2 * hp + 2, 0:2, :],
                                         start=False, stop=(hp == 2), perf_mode=DR)
                        nc.tensor.matmul(zps[:, 512:1024], w2_sb[:, hp, m, :, :],
                                         pf8[:, 2 * hp:2 * hp + 2, 2:4, :],
                                         start=False, stop=(hp == 2), perf_mode=DR)
                    mid = fin.tile([128, 1024], bf, tag="mid", bufs=2)
                    nc.scalar.activation(mid[:], zps[:], AF.Sigmoid,
                                         bias=b12_b[:, m:m + 1], scale=1.0 / SCW)
                    for o in range(4):
                        d = fin.tile([128, 256], f32, tag="fd", bufs=3)
                        nc.gpsimd.tensor_sub(d[:], pbf[:, m, o, :], ocT[:, m, o, :])
                        nc.vector.tensor_mul(d[:], d[:], mid[:, o * 256:(o + 1) * 256])
                        fo = fin.tile([128, 256], f32, tag="fout", bufs=4)
                        nc.vector.tensor_add(fo[:], d[:], ocT[:, m, o, :])
                        nc.sync.dma_start(out_d.ap()[g][o][m], fo[:])

    nc.compile()
    return nc


def _pack_weights(Wt, bt, Wd, bd, Wd3, bd3, W1, b1, W2, b2):
    def lhs_blocks(w):  # [H,H] -> [128, HC(k), HC(m), 128] of W.T
        return np.ascontiguousarray(
            w.T.reshape(HC, 128, HC, 128).transpose(1, 0, 2, 3))

    def dr_blocks(w, scale):  # [H,H] -> [128, 3(hp), HC(m), 2, 128] fp8
        blk = lhs_blocks(np.asarray(w, np.float32))  # [128, k(6), m(6), 128]
        blk = blk.reshape(128, 3, 2, HC, 128).transpose(0, 1, 3, 2, 4)
        return _clip8(np.ascontiguousarray(blk), scale)

    wt = lhs_blocks(np.asarray(Wt, np.float32)).astype(BF16)
    w1 = dr_blocks(W1, SCW)
    w2 = dr_blocks(W2, SCW2)
    wd = np.ascontiguousarray(
        np.asarray(Wd, np.float32).T.reshape(HC, 128, H).transpose(1, 0, 2)).astype(BF16)

    def wd3_block(k):  # [128, 3(hp), HC(m), 2, 128] fp8
        blk = np.ascontiguousarray(
            np.asarray(Wd3, np.float32)[:, k * H:(k + 1) * H].T
            .reshape(HC, 128, HC, 128).transpose(1, 0, 2, 3))
        blk = blk.reshape(128, 3, 2, HC, 128).transpose(0, 1, 3, 2, 4)
        return _clip8(np.ascontiguousarray(blk), SCW)

    wd3 = np.stack([wd3_block(k) for k in range(3)])
    biases = np.stack([
        np.asarray(v, np.float32).reshape(HC, 128).T
        for v in (bt, np.asarray(bd, np.float32) * SCK, bd3,
                  np.asarray(b1, np.float32) + np.asarray(b2, np.float32))
    ], axis=1)  # [128, 4, HC]
    biases = np.ascontiguousarray(biases, np.float32)
    return wt, wd, wd3, w1, w2, biases


def kernel(**inputs):
    from concourse.bass_utils import run_bass_kernel_spmd

    p = np.asarray(inputs["p"], np.float32)
    option_len = np.asarray(inputs["option_len"]).astype(np.int64)
    lens = (option_len + 1).astype(np.int64)  # [B] key lengths
    glens = lens.reshape(B // 4, 4)

    slots = _assign_groups(glens)  # [GPC][8] group ids
    slot_lens = tuple(
        tuple(int(glens[slots[g]].max(axis=0)[o]) for o in range(4))
        for g in range(GPC))

    if slot_lens not in _GRAPH_CACHE:
        _GRAPH_CACHE[slot_lens] = _build_graph(slot_lens)
    nc = _GRAPH_CACHE[slot_lens]

    wt, wd, wd3, w1, w2, biases = _pack_weights(
        inputs["Wt"], inputs["bt"], inputs["Wd"], inputs["bd"],
        inputs["Wd3"], inputs["bd3"], inputs["W1"], inputs["b1"],
        inputs["W2"], inputs["b2"])

    in_maps = []
    core_groups = []  # [core][g] -> group id
    for c in range(N_CORES):
        gids = [slots[g][c] for g in range(GPC)]
        core_groups.append(gids)
        opts = np.concatenate([np.arange(4) + 4 * gid for gid in gids])
        pc = p[opts]  # [16, S, H]
        pT = pc.transpose(0, 2, 1).reshape(GPC, 4, HC, 128, S)
        pT = np.ascontiguousarray(pT.transpose(0, 2, 3, 1, 4))  # [g, hc, p, o, s]
        maskrow = np.zeros((GPC, 1, 4, S + 1), np.float32)
        for g in range(GPC):
            for o in range(4):
                lc = int(glens[gids[g]][o])
                sl = int(slot_lens[g][o])
                maskrow[g, 0, o, lc:sl] = -30000.0
                maskrow[g, 0, o, sl] = math.log(1e-13 * (S - lc))
        in_maps.append({
            "p_bf": pT.astype(BF16),
            "p_f8": _clip8(pT, SCP),
            "maskrow": maskrow.astype(BF16),
            "wt": wt, "wd": wd, "wd3": wd3, "w1": w1, "w2": w2,
            "biases": biases,
        })

    try:
        res = run_bass_kernel_spmd(nc, in_maps, list(range(N_CORES)))
    except Exception:
        # a previously wedged device surfaces on the first execute; the
        # runtime resets it, so a single retry suffices
        res = run_bass_kernel_spmd(nc, in_maps, list(range(N_CORES)))

    out = np.empty((B, S, H), np.float32)
    for c in range(N_CORES):
        oc = res.results[c]["out"]  # [GPC, 4, HC, 128, S]
        for g in range(GPC):
            gid = core_groups[c][g]
            # [4, HC, 128, S] -> [4, S, H]
            blk = oc[g].transpose(0, 3, 1, 2).reshape(4, S, H)
            out[4 * gid: 4 * gid + 4] = blk
    return out


# revision 4
# speedup vs baseline: 1.0608x; 1.0608x over previous
"""Trainium2 Bass kernel for the AOI (attention-over-interactions) module.

Strategy (8 NeuronCores, data-parallel over question groups):
  - B=128 options = 32 self-contained groups of 4; 4 "group slots" x 8 cores.
  - Host assigns groups to slots to minimize per-slot ragged length maxima
    (same SPMD graph on all cores; per-(slot,option) key lengths baked into
    the instruction stream as max over the 8 cores at that slot).
  - Per-core masked-softmax semantics are matched to the reference exactly
    (max over masked-zeros, i.e. M = max(0, row max), and the +1e-13
    renormalization epsilon scaled by the full-softmax partition function)
    via a per-core additive mask (host-broadcast to 128 partitions, applied
    with one vector add per score tile) plus one extra "epsilon column".
  - Mixed precision: bf16 for the attention-score path, fp32 accumulation
    everywhere.
"""

import math
import sys

for _p in ("/opt/trn_rl_repo", "/opt/pypackages"):
    if _p not in sys.path:
        sys.path.append(_p)

import numpy as np
import ml_dtypes

B, S, H = 128, 256, 768
N_CORES = 8
GPC = 4           # group slots per core
HC = H // 128     # 6 h-chunks
BF16 = ml_dtypes.bfloat16

_GRAPH_CACHE = {}


def _assign_groups(glens):
    """Partition 32 groups into 4 slots of 8 minimizing sum_slot sum_o max_core len.

    Returns slots: list[4] of list[8] group ids (core c takes slots[g][c]).
    """
    rng = np.random.default_rng(0)
    n_groups = glens.shape[0]

    def cost(assign):
        c = 0
        for g in range(GPC):
            ids = assign[g]
            c += int(glens[ids].max(axis=0).sum())
        return c

    best, best_cost = None, None
    for trial in range(6):
        if trial == 0:
            order = np.argsort(-glens.sum(axis=1))
        else:
            order = rng.permutation(n_groups)
        assign = [list(order[g * 8:(g + 1) * 8]) for g in range(GPC)]
        # 2-swap hill climbing across slots
        improved = True
        cur = cost(assign)
        it = 0
        while improved and it < 60:
            improved = False
            it += 1
            for ga in range(GPC):
                for gb in range(ga + 1, GPC):
                    for ia in range(8):
                        for ib in range(8):
                            assign[ga][ia], assign[gb][ib] = assign[gb][ib], assign[ga][ia]
                            nc_ = cost(assign)
                            if nc_ < cur:
                                cur = nc_
                                improved = True
                            else:
                                assign[ga][ia], assign[gb][ib] = assign[gb][ib], assign[ga][ia]
        if best_cost is None or cur < best_cost:
            best_cost, best = cur, [list(a) for a in assign]
    return best


def _build_graph(slot_lens):
    """Build + compile the SPMD Bacc graph. slot_lens: [GPC][4] ints (1..256)."""
    import concourse.bass as bass
    import concourse.bacc as bacc
    import concourse.mybir as mybir
    import concourse.tile as tile
    from concourse.masks import make_identity

    f32 = mybir.dt.float32
    bf = mybir.dt.bfloat16
    AX = mybir.AxisListType
    AF = mybir.ActivationFunctionType

    nc = bacc.Bacc("TRN2", target_bir_lowering=False, debug=False,
                   num_devices=N_CORES)

    p_bf_d = nc.dram_tensor("p_bf", [GPC, HC, 128, 4, S], bf, kind="ExternalInput")
    mask_d = nc.dram_tensor("maskbc", [GPC, 128, 4, S + 1], bf, kind="ExternalInput")
    wt_d = nc.dram_tensor("wt", [128, HC, HC, 128], bf, kind="ExternalInput")
    wd_d = nc.dram_tensor("wd", [128, HC, H], bf, kind="ExternalInput")
    wd3_d = nc.dram_tensor("wd3", [3, 128, HC, HC, 128], bf, kind="ExternalInput")
    w1_d = nc.dram_tensor("w1", [128, HC, HC, 128], bf, kind="ExternalInput")
    w2_d = nc.dram_tensor("w2", [128, HC, HC, 128], bf, kind="ExternalInput")
    bias_d = nc.dram_tensor("biases", [128, 4, HC], f32, kind="ExternalInput")
    out_d = nc.dram_tensor("out", [GPC, 4, HC, 128, S], f32, kind="ExternalOutput")

    with tile.TileContext(nc) as tc:
        with (
            tc.tile_pool(name="const", bufs=1) as constp,
            tc.tile_pool(name="wres", bufs=1) as wres,
            tc.tile_pool(name="wstream", bufs=2) as wstream,
            tc.tile_pool(name="gin", bufs=1) as gin,
            tc.tile_pool(name="act", bufs=1) as act,
            tc.tile_pool(name="smp", bufs=2) as smp,
            tc.tile_pool(name="fin", bufs=2) as fin,
            tc.tile_pool(name="patt", bufs=4, space="PSUM") as patt,
            tc.tile_pool(name="psbig", bufs=1, space="PSUM") as psbig,
            tc.tile_pool(name="pshalf", bufs=1, space="PSUM") as pshalf,
            tc.tile_pool(name="pstr", bufs=1, space="PSUM") as pstr,
        ):
            ident_f = constp.tile([128, 128], f32)
            make_identity(nc, ident_f[:])
            ident = constp.tile([128, 128], bf)
            nc.vector.tensor_copy(ident[:], ident_f[:])
            biases = constp.tile([128, 4, HC], f32)
            nc.sync.dma_start(biases[:], bias_d.ap())
            bt_b = biases[:, 0, :]
            bd_b = biases[:, 1, :]
            bd3_b = biases[:, 2, :]
            b12_b = biases[:, 3, :]

            wt_sb = wres.tile([128, HC, HC, 128], bf)
            nc.sync.dma_start(wt_sb[:], wt_d.ap())

            # first group's inputs land before the remaining weights so the
            # tensor engine can start early
            pbf_t, msk_t = [], []
            for g in range(GPC):
                pbf_t.append(gin.tile([128, HC, 4, S], bf, tag="pbf", bufs=2,
                                      name=f"pbf{g}"))
                msk_t.append(gin.tile([128, 4, S + 1], bf, tag="msk", bufs=2,
                                      name=f"msk{g}"))

            def load_group(g):
                for hc in range(HC):
                    nc.sync.dma_start(pbf_t[g][:, hc, :, :], p_bf_d.ap()[g][hc])
                nc.sync.dma_start(msk_t[g][:], mask_d.ap()[g])

            load_group(0)

            wd_sb = wres.tile([128, HC, H], bf)
            nc.sync.dma_start(wd_sb[:], wd_d.ap())
            w1_sb = wres.tile([128, HC, HC, 128], bf)
            nc.sync.dma_start(w1_sb[:], w1_d.ap())
            w2_sb = wres.tile([128, HC, HC, 128], bf)
            nc.sync.dma_start(w2_sb[:], w2_d.ap())

            for g in range(GPC):
                L = [int(x) for x in slot_lens[g]]
                tcs = [max(1, math.ceil(l / 128)) for l in L]

                pbf, mskb = pbf_t[g], msk_t[g]
                if g + 1 < GPC:
                    load_group(g + 1)

                # ---- trans_t (feature-major, bf16): ttT[h', t] = Wt @ pT + bt
                # one extra column per option (index L[o]) zeroed so the score
                # matmuls can produce the epsilon column via accumulation
                ttT = act.tile([128, HC, 4, S + 1], bf, tag="ttT", bufs=2)
                for m in range(HC):
                    for o in range(4):
                        ps = patt.tile([128, 257], f32, tag="patt")
                        for hc in range(HC):
                            nc.tensor.matmul(
                                ps[:, 0:L[o]],
                                wt_sb[:, hc, m, :],
                                pbf[:, hc, o, 0:L[o]],
                                start=(hc == 0), stop=(hc == HC - 1),
                            )
                        nc.scalar.activation(
                            ttT[:, m, o, 0:L[o]], ps[:, 0:L[o]],
                            AF.Identity, bias=bt_b[:, m:m + 1],
                        )
                for o in range(4):
                    nc.vector.memset(ttT[:, :, o, L[o]:L[o] + 1], 0.0)

                # ---- trans_d (natural [t, h], bf16): td = pT^T @ WdT
                td = act.tile([128, 4, 2, H], bf, tag="td")
                for o in range(4):
                    for tcx in range(tcs[o]):
                        w = min(128, L[o] - tcx * 128)
                        ps = psbig.tile([128, 1024], f32, tag="psbig")
                        for hc in range(HC):
                            lhs = pbf[:, hc, o, tcx * 128: tcx * 128 + w]
                            nc.tensor.matmul(ps[0:w, 0:512], lhs, wd_sb[:, hc, 0:512],
                                             start=(hc == 0), stop=(hc == HC - 1))
                            nc.tensor.matmul(ps[0:w, 512:768], lhs, wd_sb[:, hc, 512:768],
                                             start=(hc == 0), stop=(hc == HC - 1))
                        nc.scalar.activation(td[0:w, o, tcx, :], ps[0:w, 0:768], AF.Copy)

                # stream Wd3 for this group
                wd3_sb = []
                for k in range(3):
                    t = wstream.tile([128, HC, HC, 128], bf, tag=f"wd3_{k}", bufs=1)
                    nc.sync.dma_start(t[:], wd3_d.ap()[k])
                    wd3_sb.append(t)

                ocT = act.tile([128, HC, 4, S], bf, tag="ocT")
                for half in range(2):
                    ock = act.tile([128, 3, HC, 2, S], bf, tag="ock", bufs=2)
                    for io in range(2):
                        i = half * 2 + io
                        jlist = [j for j in range(4) if j != i]
                        # ---- att scores for the 3 partners, [s_m, t] layout
                        for jr, j in enumerate(jlist):
                            lj = L[j]
                            smc = []
                            for m in range(2):
                                ps = patt.tile([128, 257], f32, tag="patt")
                                for hc in range(HC):
                                    nc.tensor.matmul(
                                        ps[:, 0:lj + 1],
                                        pbf[:, hc, i, m * 128:(m + 1) * 128],
                                        ttT[:, hc, j, 0:lj + 1],
                                        start=(hc == 0), stop=(hc == HC - 1),
                                    )
                                nc.vector.tensor_add(
                                    ps[:, 0:lj + 1], ps[:, 0:lj + 1],
                                    mskb[:, j, 0:lj + 1])
                                stats = smp.tile([128, 16], f32, tag="stats", bufs=4)
                                nc.vector.tensor_reduce(
                                    stats[:, 0:1], ps[:, 0:lj], AX.X,
                                    mybir.AluOpType.max, negate=True)
                                # -M = min(0, -max)
                                nc.vector.tensor_scalar_min(stats[:, 0:1], stats[:, 0:1], 0.0)
                                e = smp.tile([128, 257], f32, tag="e", bufs=2)
                                nc.scalar.activation(
                                    e[:, 0:lj + 1], ps[:, 0:lj + 1],
                                    AF.Exp, bias=stats[:, 0:1], scale=1.0,
                                    accum_out=stats[:, 1:2])
                                nc.vector.reciprocal(stats[:, 2:3], stats[:, 1:2])
                                sm = smp.tile([128, 256], bf, tag="sm", bufs=4)
                                nc.vector.tensor_scalar_mul(sm[:, 0:lj], e[:, 0:lj],
                                                            stats[:, 2:3])
                                smc.append(sm)
                            smT = smp.tile([128, 2, 256], bf, tag="smT", bufs=2)
                            for tcx in range(tcs[j]):
                                w = min(128, lj - tcx * 128)
                                tp = pstr.tile([128, 256], bf, tag="pstr")
                                for m in range(2):
                                    nc.tensor.matmul(
                                        tp[0:w, m * 128:(m + 1) * 128],
                                        smc[m][:, tcx * 128: tcx * 128 + w],
                                        ident[:], is_transpose=True,
                                        start=(m == 0), stop=(m == 1))
                                nc.scalar.activation(smT[0:w, tcx, :], tp[0:w, :],
                                                     AF.Copy)
                            # ---- av: avT[h', s] += td_j^T(blocks) @ smT
                            for hc in range(HC):
                                aps = patt.tile([128, 257], f32, tag="patt")
                                for tcx in range(tcs[j]):
                                    w = min(128, lj - tcx * 128)
                                    nc.tensor.matmul(
                                        aps[:, 0:256],
                                        td[0:w, j, tcx, hc * 128:(hc + 1) * 128],
                                        smT[0:w, tcx, :],
                                        start=(tcx == 0), stop=(tcx == tcs[j] - 1))
                                nc.scalar.activation(
                                    ock[:, jr, hc, io, :], aps[:, 0:256],
                                    AF.Relu, bias=bd_b[:, hc:hc + 1])

                    # ---- oc for this half (2 options): ocT = sum_k Wd3_k @ ock_k + bd3
                    for m in range(HC):
                        ops = pshalf.tile([128, 512], f32, tag="pshalf")
                        step = 0
                        for k in range(3):
                            for hc in range(HC):
                                nc.tensor.matmul(
                                    ops[:, :], wd3_sb[k][:, hc, m, :],
                                    ock[:, k, hc, :, :],
                                    start=(step == 0), stop=(step == 17))
                                step += 1
                        for oo in range(2):
                            o = half * 2 + oo
                            nc.scalar.activation(
                                ocT[:, m, o, :], ops[:, oo * 256:(oo + 1) * 256],
                                AF.Identity, bias=bd3_b[:, m:m + 1])

                # ---- mid + final output, per m-chunk
                for m in range(HC):
                    zps = psbig.tile([128, 1024], f32, tag="psbig")
                    for hc in range(HC):
                        nc.tensor.matmul(zps[:, 0:512], w1_sb[:, hc, m, :],
                                         ocT[:, hc, 0:2, :],
                                         start=(hc == 0), stop=False)
                        nc.tensor.matmul(zps[:, 512:1024], w1_sb[:, hc, m, :],
                                         ocT[:, hc, 2:4, :],
                                         start=(hc == 0), stop=False)
                    for hc in range(HC):
                        nc.tensor.matmul(zps[:, 0:512], w2_sb[:, hc, m, :],
                                         pbf[:, hc, 0:2, :],
                                         start=False, stop=(hc == HC - 1))
                        nc.tensor.matmul(zps[:, 512:1024], w2_sb[:, hc, m, :],
                                         pbf[:, hc, 2:4, :],
                                         start=False, stop=(hc == HC - 1))
                    mid = fin.tile([128, 1024], bf, tag="mid", bufs=2)
                    nc.scalar.activation(mid[:], zps[:], AF.Sigmoid,
                                         bias=b12_b[:, m:m + 1])
                    for o in range(4):
                        d = fin.tile([128, 256], f32, tag="fd", bufs=3)
                        nc.gpsimd.tensor_sub(d[:], pbf[:, m, o, :], ocT[:, m, o, :])
                        nc.vector.tensor_mul(d[:], d[:], mid[:, o * 256:(o + 1) * 256])
                        fo = fin.tile([128, 256], f32, tag="fout", bufs=4)
                        nc.vector.tensor_add(fo[:], d[:], ocT[:, m, o, :])
                        nc.sync.dma_start(out_d.ap()[g][o][m], fo[:])

    nc.compile()
    return nc


def _pack_weights(Wt, bt, Wd, bd, Wd3, bd3, W1, b1, W2, b2):
    def lhs_blocks(w):  # [H,H] -> [128, HC(k), HC(m), 128] of W.T
        return np.ascontiguousarray(
            w.T.reshape(HC, 128, HC, 128).transpose(1, 0, 2, 3))

    wt = lhs_blocks(np.asarray(Wt, np.float32)).astype(BF16)
    w1 = lhs_blocks(np.asarray(W1, np.float32)).astype(BF16)
    w2 = lhs_blocks(np.asarray(W2, np.float32)).astype(BF16)
    wd = np.ascontiguousarray(
        np.asarray(Wd, np.float32).T.reshape(HC, 128, H).transpose(1, 0, 2)).astype(BF16)

    def wd3_block(k):
        blk = np.ascontiguousarray(
            np.asarray(Wd3, np.float32)[:, k * H:(k + 1) * H].T
            .reshape(HC, 128, HC, 128).transpose(1, 0, 2, 3))
        return blk.astype(BF16)

    wd3 = np.stack([wd3_block(k) for k in range(3)])
    biases = np.stack([
        np.asarray(v, np.float32).reshape(HC, 128).T
        for v in (bt, bd, bd3, np.asarray(b1, np.float32) + np.asarray(b2, np.float32))
    ], axis=1)  # [128, 4, HC]
    biases = np.ascontiguousarray(biases, np.float32)
    return wt, wd, wd3, w1, w2, biases


def kernel(**inputs):
    from concourse.bass_utils import run_bass_kernel_spmd

    p = np.asarray(inputs["p"], np.float32)
    option_len = np.asarray(inputs["option_len"]).astype(np.int64)
    lens = (option_len + 1).astype(np.int64)  # [B] key lengths
    glens = lens.reshape(B // 4, 4)

    slots = _assign_groups(glens)  # [GPC][8] group ids
    slot_lens = tuple(
        tuple(int(glens[slots[g]].max(axis=0)[o]) for o in range(4))
        for g in range(GPC))

    if slot_lens not in _GRAPH_CACHE:
        _GRAPH_CACHE[slot_lens] = _build_graph(slot_lens)
    nc = _GRAPH_CACHE[slot_lens]

    wt, wd, wd3, w1, w2, biases = _pack_weights(
        inputs["Wt"], inputs["bt"], inputs["Wd"], inputs["bd"],
        inputs["Wd3"], inputs["bd3"], inputs["W1"], inputs["b1"],
        inputs["W2"], inputs["b2"])

    in_maps = []
    core_groups = []  # [core][g] -> group id
    for c in range(N_CORES):
        gids = [slots[g][c] for g in range(GPC)]
        core_groups.append(gids)
        opts = np.concatenate([np.arange(4) + 4 * gid for gid in gids])
        pc = p[opts]  # [16, S, H]
        pT = pc.transpose(0, 2, 1).reshape(GPC, 4, HC, 128, S)
        pT = np.ascontiguousarray(pT.transpose(0, 2, 3, 1, 4))  # [g, hc, p, o, s]
        maskrow = np.zeros((GPC, 1, 4, S + 1), np.float32)
        for g in range(GPC):
            for o in range(4):
                lc = int(glens[gids[g]][o])
                sl = int(slot_lens[g][o])
                maskrow[g, 0, o, lc:sl] = -30000.0
                maskrow[g, 0, o, sl] = math.log(1e-13 * (S - lc))
        maskbc = np.broadcast_to(maskrow, (GPC, 128, 4, S + 1))
        in_maps.append({
            "p_bf": pT.astype(BF16),
            "maskbc": np.ascontiguousarray(maskbc).astype(BF16),
            "wt": wt, "wd": wd, "wd3": wd3, "w1": w1, "w2": w2,
            "biases": biases,
        })

    try:
        res = run_bass_kernel_spmd(nc, in_maps, list(range(N_CORES)))
    except Exception:
        # a previously wedged device surfaces on the first execute; the
        # runtime resets it, so a single retry suffices
        res = run_bass_kernel_spmd(nc, in_maps, list(range(N_CORES)))

    out = np.empty((B, S, H), np.float32)
    for c in range(N_CORES):
        oc = res.results[c]["out"]  # [GPC, 4, HC, 128, S]
        for g in range(GPC):
            gid = core_groups[c][g]
            # [4, HC, 128, S] -> [4, S, H]
            blk = oc[g].transpose(0, 3, 1, 2).reshape(4, S, H)
            out[4 * gid: 4 * gid + 4] = blk
    return out


# revision 6
# speedup vs baseline: 1.2261x; 1.1558x over previous
"""Trainium2 Bass kernel for the AOI (attention-over-interactions) module.

Strategy (8 NeuronCores, data-parallel over question groups):
  - B=128 options = 32 self-contained groups of 4; 4 "group slots" x 8 cores.
  - Host assigns groups to slots to minimize per-slot ragged length maxima
    (same SPMD graph on all cores; per-(slot,option) key lengths baked into
    the instruction stream as max over the 8 cores at that slot).
  - Per-core masked-softmax semantics are matched to the reference exactly
    (max over masked-zeros, i.e. M = max(0, row max), and the +1e-13
    renormalization epsilon scaled by the full-softmax partition function)
    via a per-core additive mask (host-broadcast to 128 partitions, applied
    with one vector add per score tile) plus one extra "epsilon column".
  - Mixed precision: bf16 for the attention-score path, fp32 accumulation
    everywhere.
"""

import math
import sys

for _p in ("/opt/trn_rl_repo", "/opt/pypackages"):
    if _p not in sys.path:
        sys.path.append(_p)

import numpy as np
import ml_dtypes

B, S, H = 128, 256, 768
N_CORES = 8
GPC = 4           # group slots per core
HC = H // 128     # 6 h-chunks
BF16 = ml_dtypes.bfloat16

_GRAPH_CACHE = {}


def _assign_groups(glens):
    """Partition 32 groups into 4 slots of 8 minimizing sum_slot sum_o max_core len.

    Returns slots: list[4] of list[8] group ids (core c takes slots[g][c]).
    """
    rng = np.random.default_rng(0)
    n_groups = glens.shape[0]

    def cost(assign):
        c = 0
        for g in range(GPC):
            ids = assign[g]
            c += int(glens[ids].max(axis=0).sum())
        return c

    best, best_cost = None, None
    for trial in range(6):
        if trial == 0:
            order = np.argsort(-glens.sum(axis=1))
        else:
            order = rng.permutation(n_groups)
        assign = [list(order[g * 8:(g + 1) * 8]) for g in range(GPC)]
        # 2-swap hill climbing across slots
        improved = True
        cur = cost(assign)
        it = 0
        while improved and it < 60:
            improved = False
            it += 1
            for ga in range(GPC):
                for gb in range(ga + 1, GPC):
                    for ia in range(8):
                        for ib in range(8):
                            assign[ga][ia], assign[gb][ib] = assign[gb][ib], assign[ga][ia]
                            nc_ = cost(assign)
                            if nc_ < cur:
                                cur = nc_
                                improved = True
                            else:
                                assign[ga][ia], assign[gb][ib] = assign[gb][ib], assign[ga][ia]
        if best_cost is None or cur < best_cost:
            best_cost, best = cur, [list(a) for a in assign]
    return best


def _build_graph(slot_lens):
    """Build + compile the SPMD Bacc graph. slot_lens: [GPC][4] ints (1..256)."""
    import concourse.bass as bass
    import concourse.bacc as bacc
    import concourse.mybir as mybir
    import concourse.tile as tile
    from concourse.masks import make_identity

    f32 = mybir.dt.float32
    bf = mybir.dt.bfloat16
    AX = mybir.AxisListType
    AF = mybir.ActivationFunctionType

    nc = bacc.Bacc("TRN2", target_bir_lowering=False, debug=False,
                   num_devices=N_CORES)

    p_bf_d = nc.dram_tensor("p_bf", [GPC, HC, 128, 4, S], bf, kind="ExternalInput")
    mask_d = nc.dram_tensor("maskbc", [GPC, 128, 4, S + 1], bf, kind="ExternalInput")
    wt_d = nc.dram_tensor("wt", [128, HC, HC, 128], bf, kind="ExternalInput")
    wd_d = nc.dram_tensor("wd", [128, HC, H], bf, kind="ExternalInput")
    wd3_d = nc.dram_tensor("wd3", [3, 128, HC, HC, 128], bf, kind="ExternalInput")
    w1_d = nc.dram_tensor("w1", [128, HC, HC, 128], bf, kind="ExternalInput")
    w2_d = nc.dram_tensor("w2", [128, HC, HC, 128], bf, kind="ExternalInput")
    bias_d = nc.dram_tensor("biases", [128, 4, HC], f32, kind="ExternalInput")
    out_d = nc.dram_tensor("out", [GPC, 4, HC, 128, S], f32, kind="ExternalOutput")

    with tile.TileContext(nc) as tc:
        with (
            tc.tile_pool(name="const", bufs=1) as constp,
            tc.tile_pool(name="wres", bufs=1) as wres,
            tc.tile_pool(name="wstream", bufs=2) as wstream,
            tc.tile_pool(name="gin", bufs=1) as gin,
            tc.tile_pool(name="act", bufs=1) as act,
            tc.tile_pool(name="smp", bufs=2) as smp,
            tc.tile_pool(name="fin", bufs=2) as fin,
            tc.tile_pool(name="patt", bufs=4, space="PSUM") as patt,
            tc.tile_pool(name="psbig", bufs=1, space="PSUM") as psbig,
            tc.tile_pool(name="pshalf", bufs=1, space="PSUM") as pshalf,
            tc.tile_pool(name="pstr", bufs=1, space="PSUM") as pstr,
        ):
            ident_f = constp.tile([128, 128], f32)
            make_identity(nc, ident_f[:])
            ident = constp.tile([128, 128], bf)
            nc.vector.tensor_copy(ident[:], ident_f[:])
            biases = constp.tile([128, 4, HC], f32)
            nc.sync.dma_start(biases[:], bias_d.ap())
            bt_b = biases[:, 0, :]
            bd_b = biases[:, 1, :]
            bd3_b = biases[:, 2, :]
            b12_b = biases[:, 3, :]

            wt_sb = wres.tile([128, HC, HC, 128], bf)
            nc.sync.dma_start(wt_sb[:], wt_d.ap())

            # first group's inputs land before the remaining weights so the
            # tensor engine can start early
            pbf_t, msk_t = [], []
            for g in range(GPC):
                pbf_t.append(gin.tile([128, HC, 4, S], bf, tag="pbf", bufs=2,
                                      name=f"pbf{g}"))
                msk_t.append(gin.tile([128, 4, S + 1], bf, tag="msk", bufs=2,
                                      name=f"msk{g}"))

            def load_group(g):
                for hc in range(HC):
                    nc.sync.dma_start(pbf_t[g][:, hc, :, :], p_bf_d.ap()[g][hc])
                nc.sync.dma_start(msk_t[g][:], mask_d.ap()[g])

            load_group(0)

            wd_sb = wres.tile([128, HC, H], bf)
            nc.sync.dma_start(wd_sb[:], wd_d.ap())
            w1_sb = wres.tile([128, HC, HC, 128], bf)
            nc.sync.dma_start(w1_sb[:], w1_d.ap())
            w2_sb = wres.tile([128, HC, HC, 128], bf)
            nc.sync.dma_start(w2_sb[:], w2_d.ap())

            for g in range(GPC):
                L = [int(x) for x in slot_lens[g]]
                tcs = [max(1, math.ceil(l / 128)) for l in L]

                pbf, mskb = pbf_t[g], msk_t[g]
                if g + 1 < GPC:
                    load_group(g + 1)

                # ---- trans_t (feature-major, bf16): ttT[h', t] = Wt @ pT + bt
                # one extra column per option (index L[o]) zeroed so the score
                # matmuls can produce the epsilon column via accumulation
                ttT = act.tile([128, HC, 4, S + 1], bf, tag="ttT", bufs=2)
                for m in range(HC):
                    for o in range(4):
                        ps = patt.tile([128, 257], f32, tag="patt")
                        for hc in range(HC):
                            nc.tensor.matmul(
                                ps[:, 0:L[o]],
                                wt_sb[:, hc, m, :],
                                pbf[:, hc, o, 0:L[o]],
                                start=(hc == 0), stop=(hc == HC - 1),
                            )
                        nc.scalar.activation(
                            ttT[:, m, o, 0:L[o]], ps[:, 0:L[o]],
                            AF.Identity, bias=bt_b[:, m:m + 1],
                        )
                for o in range(4):
                    nc.vector.memset(ttT[:, :, o, L[o]:L[o] + 1], 0.0)

                # ---- trans_d (natural [t, h], bf16): td = pT^T @ WdT
                td = act.tile([128, 4, 2, H], bf, tag="td")
                for o in range(4):
                    for tcx in range(tcs[o]):
                        w = min(128, L[o] - tcx * 128)
                        ps = psbig.tile([128, 1024], f32, tag="psbig")
                        for hc in range(HC):
                            lhs = pbf[:, hc, o, tcx * 128: tcx * 128 + w]
                            nc.tensor.matmul(ps[0:w, 0:512], lhs, wd_sb[:, hc, 0:512],
                                             start=(hc == 0), stop=(hc == HC - 1))
                            nc.tensor.matmul(ps[0:w, 512:768], lhs, wd_sb[:, hc, 512:768],
                                             start=(hc == 0), stop=(hc == HC - 1))
                        nc.scalar.activation(td[0:w, o, tcx, :], ps[0:w, 0:768], AF.Copy)

                # stream Wd3 for this group
                wd3_sb = []
                for k in range(3):
                    t = wstream.tile([128, HC, HC, 128], bf, tag=f"wd3_{k}", bufs=1)
                    nc.sync.dma_start(t[:], wd3_d.ap()[k])
                    wd3_sb.append(t)

                ocT = act.tile([128, HC, 4, S], bf, tag="ocT")
                for half in range(2):
                    ock = act.tile([128, 3, HC, 2, S], bf, tag="ock", bufs=2)
                    for io in range(2):
                        i = half * 2 + io
                        jlist = [j for j in range(4) if j != i]

                        def emit_scores(j):
                            # scores + softmax chain for one partner; the
                            # vector/scalar chain overlaps the next partner's
                            # score matmuls (software pipelining)
                            lj = L[j]
                            smc = []
                            for m in range(2):
                                ps = patt.tile([128, 257], f32, tag="patt",
                                               name="ps")
                                for hc in range(HC):
                                    nc.tensor.matmul(
                                        ps[:, 0:lj + 1],
                                        pbf[:, hc, i, m * 128:(m + 1) * 128],
                                        ttT[:, hc, j, 0:lj + 1],
                                        start=(hc == 0), stop=(hc == HC - 1),
                                    )
                                nc.vector.tensor_add(
                                    ps[:, 0:lj + 1], ps[:, 0:lj + 1],
                                    mskb[:, j, 0:lj + 1])
                                stats = smp.tile([128, 16], f32, tag="stats",
                                                 bufs=8, name="stats")
                                nc.vector.tensor_reduce(
                                    stats[:, 0:1], ps[:, 0:lj], AX.X,
                                    mybir.AluOpType.max, negate=True)
                                # -M = min(0, -max)
                                nc.vector.tensor_scalar_min(stats[:, 0:1], stats[:, 0:1], 0.0)
                                e = smp.tile([128, 257], f32, tag="e", bufs=4,
                                             name="e")
                                nc.scalar.activation(
                                    e[:, 0:lj + 1], ps[:, 0:lj + 1],
                                    AF.Exp, bias=stats[:, 0:1], scale=1.0,
                                    accum_out=stats[:, 1:2])
                                nc.vector.reciprocal(stats[:, 2:3], stats[:, 1:2])
                                sm = smp.tile([128, 256], bf, tag="sm", bufs=6,
                                              name="sm")
                                nc.vector.tensor_scalar_mul(sm[:, 0:lj], e[:, 0:lj],
                                                            stats[:, 2:3])
                                smc.append(sm)
                            return smc

                        def emit_transpose(j, smc):
                            lj = L[j]
                            smT = smp.tile([128, 2, 256], bf, tag="smT", bufs=3,
                                           name="smT")
                            for tcx in range(tcs[j]):
                                w = min(128, lj - tcx * 128)
                                tp = pstr.tile([128, 256], bf, tag="pstr",
                                               name="tp")
                                for m in range(2):
                                    nc.tensor.matmul(
                                        tp[0:w, m * 128:(m + 1) * 128],
                                        smc[m][:, tcx * 128: tcx * 128 + w],
                                        ident[:], is_transpose=True,
                                        start=(m == 0), stop=(m == 1))
                                nc.scalar.activation(smT[0:w, tcx, :], tp[0:w, :],
                                                     AF.Copy)
                            return smT

                        def emit_av(j, jr, smT):
                            # av: avT[h', s] += td_j^T(blocks) @ smT
                            lj = L[j]
                            for hc in range(HC):
                                aps = patt.tile([128, 257], f32, tag="patt",
                                                name="aps")
                                for tcx in range(tcs[j]):
                                    w = min(128, lj - tcx * 128)
                                    nc.tensor.matmul(
                                        aps[:, 0:256],
                                        td[0:w, j, tcx, hc * 128:(hc + 1) * 128],
                                        smT[0:w, tcx, :],
                                        start=(tcx == 0), stop=(tcx == tcs[j] - 1))
                                nc.scalar.activation(
                                    ock[:, jr, hc, io, :], aps[:, 0:256],
                                    AF.Relu, bias=bd_b[:, hc:hc + 1])

                        smc0 = emit_scores(jlist[0])
                        smc1 = emit_scores(jlist[1])
                        smT0 = emit_transpose(jlist[0], smc0)
                        emit_av(jlist[0], 0, smT0)
                        smc2 = emit_scores(jlist[2])
                        smT1 = emit_transpose(jlist[1], smc1)
                        emit_av(jlist[1], 1, smT1)
                        smT2 = emit_transpose(jlist[2], smc2)
                        emit_av(jlist[2], 2, smT2)

                    # ---- oc for this half (2 options): ocT = sum_k Wd3_k @ ock_k + bd3
                    for m in range(HC):
                        ops = pshalf.tile([128, 512], f32, tag="pshalf")
                        step = 0
                        for k in range(3):
                            for hc in range(HC):
                                nc.tensor.matmul(
                                    ops[:, :], wd3_sb[k][:, hc, m, :],
                                    ock[:, k, hc, :, :],
                                    start=(step == 0), stop=(step == 17))
                                step += 1
                        for oo in range(2):
                            o = half * 2 + oo
                            nc.scalar.activation(
                                ocT[:, m, o, :], ops[:, oo * 256:(oo + 1) * 256],
                                AF.Identity, bias=bd3_b[:, m:m + 1])

                # ---- mid + final output, per m-chunk
                for m in range(HC):
                    # w2 @ p first: no dependency on the oc stage, so the PE
                    # rolls straight into the final stage while the last ocT
                    # activations drain
                    zps = psbig.tile([128, 1024], f32, tag="psbig")
                    for hc in range(HC):
                        nc.tensor.matmul(zps[:, 0:512], w2_sb[:, hc, m, :],
                                         pbf[:, hc, 0:2, :],
                                         start=(hc == 0), stop=False)
                        nc.tensor.matmul(zps[:, 512:1024], w2_sb[:, hc, m, :],
                                         pbf[:, hc, 2:4, :],
                                         start=(hc == 0), stop=False)
                    for hc in range(HC):
                        nc.tensor.matmul(zps[:, 0:512], w1_sb[:, hc, m, :],
                                         ocT[:, hc, 0:2, :],
                                         start=False, stop=(hc == HC - 1))
                        nc.tensor.matmul(zps[:, 512:1024], w1_sb[:, hc, m, :],
                                         ocT[:, hc, 2:4, :],
                                         start=False, stop=(hc == HC - 1))
                    mid = fin.tile([128, 1024], bf, tag="mid", bufs=2)
                    nc.scalar.activation(mid[:], zps[:], AF.Sigmoid,
                                         bias=b12_b[:, m:m + 1])
                    for o in range(4):
                        d = fin.tile([128, 256], f32, tag="fd", bufs=3)
                        nc.gpsimd.tensor_sub(d[:], pbf[:, m, o, :], ocT[:, m, o, :])
                        nc.vector.tensor_mul(d[:], d[:], mid[:, o * 256:(o + 1) * 256])
                        fo = fin.tile([128, 256], f32, tag="fout", bufs=4)
                        nc.vector.tensor_add(fo[:], d[:], ocT[:, m, o, :])
                        nc.sync.dma_start(out_d.ap()[g][o][m], fo[:])

    nc.compile()
    return nc


def _pack_weights(Wt, bt, Wd, bd, Wd3, bd3, W1, b1, W2, b2):
    def lhs_blocks(w):  # [H,H] -> [128, HC(k), HC(m), 128] of W.T
        return np.ascontiguousarray(
            w.T.reshape(HC, 128, HC, 128).transpose(1, 0, 2, 3))

    wt = lhs_blocks(np.asarray(Wt, np.float32)).astype(BF16)
    w1 = lhs_blocks(np.asarray(W1, np.float32)).astype(BF16)
    w2 = lhs_blocks(np.asarray(W2, np.float32)).astype(BF16)
    wd = np.ascontiguousarray(
        np.asarray(Wd, np.float32).T.reshape(HC, 128, H).transpose(1, 0, 2)).astype(BF16)

    def wd3_block(k):
        blk = np.ascontiguousarray(
            np.asarray(Wd3, np.float32)[:, k * H:(k + 1) * H].T
            .reshape(HC, 128, HC, 128).transpose(1, 0, 2, 3))
        return blk.astype(BF16)

    wd3 = np.stack([wd3_block(k) for k in range(3)])
    biases = np.stack([
        np.asarray(v, np.float32).reshape(HC, 128).T
        for v in (bt, bd, bd3, np.asarray(b1, np.float32) + np.asarray(b2, np.float32))
    ], axis=1)  # [128, 4, HC]
    biases = np.ascontiguousarray(biases, np.float32)
    return wt, wd, wd3, w1, w2, biases


def kernel(**inputs):
    from concourse.bass_utils import run_bass_kernel_spmd

    p = np.asarray(inputs["p"], np.float32)
    option_len = np.asarray(inputs["option_len"]).astype(np.int64)
    lens = (option_len + 1).astype(np.int64)  # [B] key lengths
    glens = lens.reshape(B // 4, 4)

    slots = _assign_groups(glens)  # [GPC][8] group ids
    slot_lens = tuple(
        tuple(int(glens[slots[g]].max(axis=0)[o]) for o in range(4))
        for g in range(GPC))

    if slot_lens not in _GRAPH_CACHE:
        _GRAPH_CACHE[slot_lens] = _build_graph(slot_lens)
    nc = _GRAPH_CACHE[slot_lens]

    wt, wd, wd3, w1, w2, biases = _pack_weights(
        inputs["Wt"], inputs["bt"], inputs["Wd"], inputs["bd"],
        inputs["Wd3"], inputs["bd3"], inputs["W1"], inputs["b1"],
        inputs["W2"], inputs["b2"])

    in_maps = []
    core_groups = []  # [core][g] -> group id
    for c in range(N_CORES):
        gids = [slots[g][c] for g in range(GPC)]
        core_groups.append(gids)
        opts = np.concatenate([np.arange(4) + 4 * gid for gid in gids])
        pc = p[opts]  # [16, S, H]
        pT = pc.transpose(0, 2, 1).reshape(GPC, 4, HC, 128, S)
        pT = np.ascontiguousarray(pT.transpose(0, 2, 3, 1, 4))  # [g, hc, p, o, s]
        maskrow = np.zeros((GPC, 1, 4, S + 1), np.float32)
        for g in range(GPC):
            for o in range(4):
                lc = int(glens[gids[g]][o])
                sl = int(slot_lens[g][o])
                maskrow[g, 0, o, lc:sl] = -30000.0
                maskrow[g, 0, o, sl] = math.log(1e-13 * (S - lc))
        maskbc = np.broadcast_to(maskrow, (GPC, 128, 4, S + 1))
        in_maps.append({
            "p_bf": pT.astype(BF16),
            "maskbc": np.ascontiguousarray(maskbc).astype(BF16),
            "wt": wt, "wd": wd, "wd3": wd3, "w1": w1, "w2": w2,
            "biases": biases,
        })

    try:
        res = run_bass_kernel_spmd(nc, in_maps, list(range(N_CORES)))
    except Exception:
        # a previously wedged device surfaces on the first execute; the
        # runtime resets it, so a single retry suffices
        res = run_bass_kernel_spmd(nc, in_maps, list(range(N_CORES)))

    out = np.empty((B, S, H), np.float32)
    for c in range(N_CORES):
        oc = res.results[c]["out"]  # [GPC, 4, HC, 128, S]
        for g in range(GPC):
            gid = core_groups[c][g]
            # [4, HC, 128, S] -> [4, S, H]
            blk = oc[g].transpose(0, 3, 1, 2).reshape(4, S, H)
            out[4 * gid: 4 * gid + 4] = blk
    return out


# revision 15
# speedup vs baseline: 1.3503x; 1.1013x over previous
"""Trainium2 Bass kernel for the AOI (attention-over-interactions) module.

Strategy (8 NeuronCores, data-parallel over question groups):
  - B=128 options = 32 self-contained groups of 4; 4 "group slots" x 8 cores.
  - Host assigns groups to slots to minimize per-slot ragged length maxima
    (same SPMD graph on all cores; per-(slot,option) key lengths baked into
    the instruction stream as max over the 8 cores at that slot).
  - Per-core masked-softmax semantics are matched to the reference exactly
    (max over masked-zeros, i.e. M = max(0, row max), and the +1e-13
    renormalization epsilon scaled by the full-softmax partition function)
    via a per-core additive mask (host-broadcast to 128 partitions, applied
    with one vector add per score tile) plus one extra "epsilon column".
  - Mixed precision: bf16 for the attention-score path, fp32 accumulation
    everywhere.
"""

import math
import sys

for _p in ("/opt/trn_rl_repo", "/opt/pypackages"):
    if _p not in sys.path:
        sys.path.append(_p)

import numpy as np
import ml_dtypes

B, S, H = 128, 256, 768
N_CORES = 8
GPC = 4           # group slots per core
HC = H // 128     # 6 h-chunks
BF16 = ml_dtypes.bfloat16
FP8 = ml_dtypes.float8_e4m3   # TRN float8e4; max 240, overflow -> inf
SCW = 2048.0      # fp8 scale for Wd3
SCK = 64.0        # fp8 scale for ock


def _clip8(x, scale):
    return np.clip(np.asarray(x, np.float32) * scale, -224.0, 224.0).astype(FP8)

_GRAPH_CACHE = {}


def _assign_groups(glens):
    """Partition 32 groups into 4 slots of 8 minimizing sum_slot sum_o max_core len.

    Returns slots: list[4] of list[8] group ids (core c takes slots[g][c]).
    """
    rng = np.random.default_rng(0)
    n_groups = glens.shape[0]

    def cost(assign):
        c = 0
        for g in range(GPC):
            ids = assign[g]
            c += int(glens[ids].max(axis=0).sum())
        return c

    best, best_cost = None, None
    for trial in range(6):
        if trial == 0:
            order = np.argsort(-glens.sum(axis=1))
        else:
            order = rng.permutation(n_groups)
        assign = [list(order[g * 8:(g + 1) * 8]) for g in range(GPC)]
        # 2-swap hill climbing across slots
        improved = True
        cur = cost(assign)
        it = 0
        while improved and it < 60:
            improved = False
            it += 1
            for ga in range(GPC):
                for gb in range(ga + 1, GPC):
                    for ia in range(8):
                        for ib in range(8):
                            assign[ga][ia], assign[gb][ib] = assign[gb][ib], assign[ga][ia]
                            nc_ = cost(assign)
                            if nc_ < cur:
                                cur = nc_
                                improved = True
                            else:
                                assign[ga][ia], assign[gb][ib] = assign[gb][ib], assign[ga][ia]
        if best_cost is None or cur < best_cost:
            best_cost, best = cur, [list(a) for a in assign]
    return best


def _build_graph(slot_lens):
    """Build + compile the SPMD Bacc graph. slot_lens: [GPC][4] ints (1..256)."""
    import concourse.bass as bass
    import concourse.bacc as bacc
    import concourse.mybir as mybir
    import concourse.tile as tile
    from concourse.masks import make_identity

    f32 = mybir.dt.float32
    bf = mybir.dt.bfloat16
    f8 = mybir.dt.float8e4
    AX = mybir.AxisListType
    AF = mybir.ActivationFunctionType
    DR = mybir.MatmulPerfMode.DoubleRow

    nc = bacc.Bacc("TRN2", target_bir_lowering=False, debug=False,
                   num_devices=N_CORES)

    p_bf_d = nc.dram_tensor("p_bf", [GPC, HC, 128, 4, S], bf, kind="ExternalInput")
    mask_d = nc.dram_tensor("maskbc", [GPC, 128, 4, S + 1], bf, kind="ExternalInput")
    wt_d = nc.dram_tensor("wt", [128, HC, HC, 128], bf, kind="ExternalInput")
    wd_d = nc.dram_tensor("wd", [128, HC, H], bf, kind="ExternalInput")
    wd3_d = nc.dram_tensor("wd3", [3, 128, 3, HC, 2, 128], f8, kind="ExternalInput")
    w1_d = nc.dram_tensor("w1", [128, HC, HC, 128], bf, kind="ExternalInput")
    w2_d = nc.dram_tensor("w2", [128, HC, HC, 128], bf, kind="ExternalInput")
    bias_d = nc.dram_tensor("biases", [128, 4, HC], f32, kind="ExternalInput")
    out_d = nc.dram_tensor("out", [GPC, 4, HC, 128, S], f32, kind="ExternalOutput")

    with tile.TileContext(nc) as tc:
        with (
            tc.tile_pool(name="const", bufs=1) as constp,
            tc.tile_pool(name="wres", bufs=1) as wres,
            tc.tile_pool(name="wstream", bufs=2) as wstream,
            tc.tile_pool(name="gin", bufs=1) as gin,
            tc.tile_pool(name="act", bufs=1) as act,
            tc.tile_pool(name="smp", bufs=2) as smp,
            tc.tile_pool(name="fin", bufs=2) as fin,
            tc.tile_pool(name="patt", bufs=4, space="PSUM") as patt,
            tc.tile_pool(name="psbig", bufs=1, space="PSUM") as psbig,
            tc.tile_pool(name="pshalf", bufs=1, space="PSUM") as pshalf,
            tc.tile_pool(name="pstr", bufs=1, space="PSUM") as pstr,
        ):
            ident_f = constp.tile([128, 128], f32)
            make_identity(nc, ident_f[:])
            ident = constp.tile([128, 128], bf)
            nc.vector.tensor_copy(ident[:], ident_f[:])
            biases = constp.tile([128, 4, HC], f32)
            nc.sync.dma_start(biases[:], bias_d.ap())
            bt_b = biases[:, 0, :]
            bd_b = biases[:, 1, :]
            bd3_b = biases[:, 2, :]
            b12_b = biases[:, 3, :]

            wt_sb = wres.tile([128, HC, HC, 128], bf)
            nc.sync.dma_start(wt_sb[:], wt_d.ap())

            # first group's inputs land before the remaining weights so the
            # tensor engine can start early
            pbf_t, msk_t = [], []
            for g in range(GPC):
                pbf_t.append(gin.tile([128, HC, 4, S], bf, tag="pbf", bufs=2,
                                      name=f"pbf{g}"))
                msk_t.append(gin.tile([128, 4, S + 1], bf, tag="msk", bufs=2,
                                      name=f"msk{g}"))

            def load_group(g):
                for hc in range(HC):
                    nc.sync.dma_start(pbf_t[g][:, hc, :, :], p_bf_d.ap()[g][hc])
                nc.sync.dma_start(msk_t[g][:], mask_d.ap()[g])

            load_group(0)

            wd_sb = wres.tile([128, HC, H], bf)
            nc.sync.dma_start(wd_sb[:], wd_d.ap())
            w1_sb = wres.tile([128, HC, HC, 128], bf)
            nc.sync.dma_start(w1_sb[:], w1_d.ap())
            w2_sb = wres.tile([128, HC, HC, 128], bf)
            nc.sync.dma_start(w2_sb[:], w2_d.ap())

            for g in range(GPC):
                L = [int(x) for x in slot_lens[g]]
                tcs = [max(1, math.ceil(l / 128)) for l in L]

                pbf, mskb = pbf_t[g], msk_t[g]
                if g + 1 < GPC:
                    load_group(g + 1)

                # ---- trans_t (feature-major, bf16): ttT[h', t] = Wt @ pT + bt
                # one extra column per option (index L[o]) zeroed so the score
                # matmuls can produce the epsilon column via accumulation
                ttT = act.tile([128, HC, 4, S + 1], bf, tag="ttT", bufs=2)
                for m in range(HC):
                    for o in range(4):
                        ps = patt.tile([128, 257], f32, tag="patt")
                        for hc in range(HC):
                            nc.tensor.matmul(
                                ps[:, 0:L[o]],
                                wt_sb[:, hc, m, :],
                                pbf[:, hc, o, 0:L[o]],
                                start=(hc == 0), stop=(hc == HC - 1),
                            )
                        nc.scalar.activation(
                            ttT[:, m, o, 0:L[o]], ps[:, 0:L[o]],
                            AF.Identity, bias=bt_b[:, m:m + 1],
                        )
                for o in range(4):
                    nc.vector.memset(ttT[:, :, o, L[o]:L[o] + 1], 0.0)

                # ---- trans_d (natural [t, h], bf16): td = pT^T @ WdT
                td = act.tile([128, 4, 2, H], bf, tag="td")
                for o in range(4):
                    for tcx in range(tcs[o]):
                        w = min(128, L[o] - tcx * 128)
                        ps = psbig.tile([128, 1024], f32, tag="psbig")
                        for hc in range(HC):
                            lhs = pbf[:, hc, o, tcx * 128: tcx * 128 + w]
                            nc.tensor.matmul(ps[0:w, 0:512], lhs, wd_sb[:, hc, 0:512],
                                             start=(hc == 0), stop=(hc == HC - 1))
                            nc.tensor.matmul(ps[0:w, 512:768], lhs, wd_sb[:, hc, 512:768],
                                             start=(hc == 0), stop=(hc == HC - 1))
                        nc.scalar.activation(td[0:w, o, tcx, :], ps[0:w, 0:768], AF.Copy)

                # stream Wd3 for this group
                wd3_sb = []
                for k in range(3):
                    t = wstream.tile([128, 3, HC, 2, 128], f8, tag=f"wd3_{k}", bufs=1)
                    nc.sync.dma_start(t[:], wd3_d.ap()[k])
                    wd3_sb.append(t)

                ocT = act.tile([128, HC, 4, S], bf, tag="ocT")
                for half in range(2):
                    ock = act.tile([128, 3, HC, 2, S], f8, tag="ock", bufs=2)
                    for io in range(2):
                        i = half * 2 + io
                        jlist = [j for j in range(4) if j != i]

                        def emit_scores(j):
                            # scores + softmax chain for one partner; the
                            # vector/scalar chain overlaps the next partner's
                            # score matmuls (software pipelining)
                            lj = L[j]
                            smc = []
                            for m in range(2):
                                ps = patt.tile([128, 257], f32, tag="patt",
                                               name="ps")
                                for hc in range(HC):
                                    nc.tensor.matmul(
                                        ps[:, 0:lj + 1],
                                        pbf[:, hc, i, m * 128:(m + 1) * 128],
                                        ttT[:, hc, j, 0:lj + 1],
                                        start=(hc == 0), stop=(hc == HC - 1),
                                    )
                                nc.vector.tensor_add(
                                    ps[:, 0:lj + 1], ps[:, 0:lj + 1],
                                    mskb[:, j, 0:lj + 1])
                                stats = smp.tile([128, 16], f32, tag="stats",
                                                 bufs=8, name="stats")
                                nc.vector.tensor_reduce(
                                    stats[:, 0:1], ps[:, 0:lj], AX.X,
                                    mybir.AluOpType.max, negate=True)
                                # -M = min(0, -max)
                                nc.vector.tensor_scalar_min(stats[:, 0:1], stats[:, 0:1], 0.0)
                                e = smp.tile([128, 257], f32, tag="e", bufs=4,
                                             name="e")
                                nc.scalar.activation(
                                    e[:, 0:lj + 1], ps[:, 0:lj + 1],
                                    AF.Exp, bias=stats[:, 0:1], scale=1.0,
                                    accum_out=stats[:, 1:2])
                                nc.vector.reciprocal(stats[:, 2:3], stats[:, 1:2])
                                sm = smp.tile([128, 256], bf, tag="sm", bufs=6,
                                              name="sm")
                                nc.vector.tensor_scalar_mul(sm[:, 0:lj], e[:, 0:lj],
                                                            stats[:, 2:3])
                                smc.append(sm)
                            return smc

                        def emit_transpose(j, smc):
                            lj = L[j]
                            smT = smp.tile([128, 2, 256], bf, tag="smT", bufs=3,
                                           name="smT")
                            for tcx in range(tcs[j]):
                                w = min(128, lj - tcx * 128)
                                tp = pstr.tile([128, 256], bf, tag="pstr",
                                               name="tp")
                                for m in range(2):
                                    nc.tensor.matmul(
                                        tp[0:w, m * 128:(m + 1) * 128],
                                        smc[m][:, tcx * 128: tcx * 128 + w],
                                        ident[:], is_transpose=True,
                                        start=(m == 0), stop=(m == 1))
                                nc.scalar.activation(smT[0:w, tcx, :], tp[0:w, :],
                                                     AF.Copy)
                            return smT

                        def emit_av(j, jr, smT):
                            # av: avT[h', s] += td_j^T(blocks) @ smT
                            lj = L[j]
                            for hc in range(HC):
                                aps = patt.tile([128, 257], f32, tag="patt",
                                                name="aps")
                                for tcx in range(tcs[j]):
                                    w = min(128, lj - tcx * 128)
                                    nc.tensor.matmul(
                                        aps[:, 0:256],
                                        td[0:w, j, tcx, hc * 128:(hc + 1) * 128],
                                        smT[0:w, tcx, :],
                                        start=(tcx == 0), stop=(tcx == tcs[j] - 1))
                                nc.scalar.activation(
                                    ock[:, jr, hc, io, :], aps[:, 0:256],
                                    AF.Relu, bias=bd_b[:, hc:hc + 1], scale=SCK)

                        smc0 = emit_scores(jlist[0])
                        smc1 = emit_scores(jlist[1])
                        smT0 = emit_transpose(jlist[0], smc0)
                        emit_av(jlist[0], 0, smT0)
                        smc2 = emit_scores(jlist[2])
                        smT1 = emit_transpose(jlist[1], smc1)
                        emit_av(jlist[1], 1, smT1)
                        smT2 = emit_transpose(jlist[2], smc2)
                        emit_av(jlist[2], 2, smT2)

                    # ---- oc for this half (2 options): ocT = sum_k Wd3_k @ ock_k + bd3
                    for m in range(HC):
                        ops = pshalf.tile([128, 512], f32, tag="pshalf")
                        step = 0
                        for k in range(3):
                            for hp in range(3):
                                nc.tensor.matmul(
                                    ops[:, :], wd3_sb[k][:, hp, m, :, :],
                                    ock[:, k, 2 * hp:2 * hp + 2, :, :],
                                    start=(step == 0), stop=(step == 8),
                                    perf_mode=DR)
                                step += 1
                        for oo in range(2):
                            o = half * 2 + oo
                            nc.scalar.activation(
                                ocT[:, m, o, :], ops[:, oo * 256:(oo + 1) * 256],
                                AF.Identity, bias=bd3_b[:, m:m + 1],
                                scale=1.0 / (SCK * SCW))

                # ---- mid + final output, per m-chunk
                for m in range(HC):
                    # w2 @ p first: no dependency on the oc stage, so the PE
                    # rolls straight into the final stage while the last ocT
                    # activations drain
                    zps = psbig.tile([128, 1024], f32, tag="psbig")
                    for hc in range(HC):
                        nc.tensor.matmul(zps[:, 0:512], w2_sb[:, hc, m, :],
                                         pbf[:, hc, 0:2, :],
                                         start=(hc == 0), stop=False)
                        nc.tensor.matmul(zps[:, 512:1024], w2_sb[:, hc, m, :],
                                         pbf[:, hc, 2:4, :],
                                         start=(hc == 0), stop=False)
                    for hc in range(HC):
                        nc.tensor.matmul(zps[:, 0:512], w1_sb[:, hc, m, :],
                                         ocT[:, hc, 0:2, :],
                                         start=False, stop=(hc == HC - 1))
                        nc.tensor.matmul(zps[:, 512:1024], w1_sb[:, hc, m, :],
                                         ocT[:, hc, 2:4, :],
                                         start=False, stop=(hc == HC - 1))
                    mid = fin.tile([128, 1024], bf, tag="mid", bufs=2)
                    nc.scalar.activation(mid[:], zps[:], AF.Sigmoid,
                                         bias=b12_b[:, m:m + 1])
                    for o in range(4):
                        d = fin.tile([128, 256], f32, tag="fd", bufs=3)
                        nc.gpsimd.tensor_sub(d[:], pbf[:, m, o, :], ocT[:, m, o, :])
                        nc.vector.tensor_mul(d[:], d[:], mid[:, o * 256:(o + 1) * 256])
                        fo = fin.tile([128, 256], f32, tag="fout", bufs=4)
                        nc.vector.tensor_add(fo[:], d[:], ocT[:, m, o, :])
                        nc.sync.dma_start(out_d.ap()[g][o][m], fo[:])

    nc.compile()
    return nc


def _pack_weights(Wt, bt, Wd, bd, Wd3, bd3, W1, b1, W2, b2):
    def lhs_blocks(w):  # [H,H] -> [128, HC(k), HC(m), 128] of W.T
        return np.ascontiguousarray(
            w.T.reshape(HC, 128, HC, 128).transpose(1, 0, 2, 3))

    wt = lhs_blocks(np.asarray(Wt, np.float32)).astype(BF16)
    w1 = lhs_blocks(np.asarray(W1, np.float32)).astype(BF16)
    w2 = lhs_blocks(np.asarray(W2, np.float32)).astype(BF16)
    wd = np.ascontiguousarray(
        np.asarray(Wd, np.float32).T.reshape(HC, 128, H).transpose(1, 0, 2)).astype(BF16)

    def wd3_block(k):  # [128, 3(hp), HC(m), 2, 128] fp8, DoubleRow pairing
        blk = np.ascontiguousarray(
            np.asarray(Wd3, np.float32)[:, k * H:(k + 1) * H].T
            .reshape(HC, 128, HC, 128).transpose(1, 0, 2, 3))
        blk = blk.reshape(128, 3, 2, HC, 128).transpose(0, 1, 3, 2, 4)
        return _clip8(np.ascontiguousarray(blk), SCW)

    wd3 = np.stack([wd3_block(k) for k in range(3)])
    biases = np.stack([
        np.asarray(v, np.float32).reshape(HC, 128).T
        for v in (bt, np.asarray(bd, np.float32) * SCK, bd3,
                  np.asarray(b1, np.float32) + np.asarray(b2, np.float32))
    ], axis=1)  # [128, 4, HC]
    biases = np.ascontiguousarray(biases, np.float32)
    return wt, wd, wd3, w1, w2, biases


def kernel(**inputs):
    from concourse.bass_utils import run_bass_kernel_spmd

    p = np.asarray(inputs["p"], np.float32)
    option_len = np.asarray(inputs["option_len"]).astype(np.int64)
    lens = (option_len + 1).astype(np.int64)  # [B] key lengths
    glens = lens.reshape(B // 4, 4)

    slots = _assign_groups(glens)  # [GPC][8] group ids
    slot_lens = tuple(
        tuple(int(glens[slots[g]].max(axis=0)[o]) for o in range(4))
        for g in range(GPC))

    if slot_lens not in _GRAPH_CACHE:
        _GRAPH_CACHE[slot_lens] = _build_graph(slot_lens)
    nc = _GRAPH_CACHE[slot_lens]

    wt, wd, wd3, w1, w2, biases = _pack_weights(
        inputs["Wt"], inputs["bt"], inputs["Wd"], inputs["bd"],
        inputs["Wd3"], inputs["bd3"], inputs["W1"], inputs["b1"],
        inputs["W2"], inputs["b2"])

    in_maps = []
    core_groups = []  # [core][g] -> group id
    for c in range(N_CORES):
        gids = [slots[g][c] for g in range(GPC)]
        core_groups.append(gids)
        opts = np.concatenate([np.arange(4) + 4 * gid for gid in gids])
        pc = p[opts]  # [16, S, H]
        pT = pc.transpose(0, 2, 1).reshape(GPC, 4, HC, 128, S)
        pT = np.ascontiguousarray(pT.transpose(0, 2, 3, 1, 4))  # [g, hc, p, o, s]
        maskrow = np.zeros((GPC, 1, 4, S + 1), np.float32)
        for g in range(GPC):
            for o in range(4):
                lc = int(glens[gids[g]][o])
                sl = int(slot_lens[g][o])
                maskrow[g, 0, o, lc:sl] = -30000.0
                maskrow[g, 0, o, sl] = math.log(1e-13 * (S - lc))
        maskbc = np.broadcast_to(maskrow, (GPC, 128, 4, S + 1))
        in_maps.append({
            "p_bf": pT.astype(BF16),
            "maskbc": np.ascontiguousarray(maskbc).astype(BF16),
            "wt": wt, "wd": wd, "wd3": wd3, "w1": w1, "w2": w2,
            "biases": biases,
        })

    try:
        res = run_bass_kernel_spmd(nc, in_maps, list(range(N_CORES)))
    except Exception:
        # a previously wedged device surfaces on the first execute; the
        # runtime resets it, so a single retry suffices
        res = run_bass_kernel_spmd(nc, in_maps, list(range(N_CORES)))

    out = np.empty((B, S, H), np.float32)
    for c in range(N_CORES):
        oc = res.results[c]["out"]  # [GPC, 4, HC, 128, S]
        for g in range(GPC):
            gid = core_groups[c][g]
            # [4, HC, 128, S] -> [4, S, H]
            blk = oc[g].transpose(0, 3, 1, 2).reshape(4, S, H)
            out[4 * gid: 4 * gid + 4] = blk
    return out


# revision 16
# speedup vs baseline: 1.3648x; 1.0108x over previous
"""Trainium2 Bass kernel for the AOI (attention-over-interactions) module.

Strategy (8 NeuronCores, data-parallel over question groups):
  - B=128 options = 32 self-contained groups of 4; 4 "group slots" x 8 cores.
  - Host assigns groups to slots to minimize per-slot ragged length maxima
    (same SPMD graph on all cores; per-(slot,option) key lengths baked into
    the instruction stream as max over the 8 cores at that slot).
  - Per-core masked-softmax semantics are matched to the reference exactly
    (max over masked-zeros, i.e. M = max(0, row max), and the +1e-13
    renormalization epsilon scaled by the full-softmax partition function)
    via a per-core additive mask (host-broadcast to 128 partitions, applied
    with one vector add per score tile) plus one extra "epsilon column".
  - Mixed precision: bf16 for the attention-score path, fp32 accumulation
    everywhere.
"""

import math
import sys

for _p in ("/opt/trn_rl_repo", "/opt/pypackages"):
    if _p not in sys.path:
        sys.path.append(_p)

import numpy as np
import ml_dtypes

B, S, H = 128, 256, 768
N_CORES = 8
GPC = 4           # group slots per core
HC = H // 128     # 6 h-chunks
BF16 = ml_dtypes.bfloat16
FP8 = ml_dtypes.float8_e4m3   # TRN float8e4; max 240, overflow -> inf
SCW = 2048.0      # fp8 scale for Wd3
SCK = 64.0        # fp8 scale for ock


def _clip8(x, scale):
    return np.clip(np.asarray(x, np.float32) * scale, -224.0, 224.0).astype(FP8)

_GRAPH_CACHE = {}


def _assign_groups(glens):
    """Partition 32 groups into 4 slots of 8 minimizing sum_slot sum_o max_core len.

    Returns slots: list[4] of list[8] group ids (core c takes slots[g][c]).
    """
    rng = np.random.default_rng(0)
    n_groups = glens.shape[0]

    def cost(assign):
        c = 0
        for g in range(GPC):
            ids = assign[g]
            c += int(glens[ids].max(axis=0).sum())
        return c

    best, best_cost = None, None
    for trial in range(6):
        if trial == 0:
            order = np.argsort(-glens.sum(axis=1))
        else:
            order = rng.permutation(n_groups)
        assign = [list(order[g * 8:(g + 1) * 8]) for g in range(GPC)]
        # 2-swap hill climbing across slots
        improved = True
        cur = cost(assign)
        it = 0
        while improved and it < 60:
            improved = False
            it += 1
            for ga in range(GPC):
                for gb in range(ga + 1, GPC):
                    for ia in range(8):
                        for ib in range(8):
                            assign[ga][ia], assign[gb][ib] = assign[gb][ib], assign[ga][ia]
                            nc_ = cost(assign)
                            if nc_ < cur:
                                cur = nc_
                                improved = True
                            else:
                                assign[ga][ia], assign[gb][ib] = assign[gb][ib], assign[ga][ia]
        if best_cost is None or cur < best_cost:
            best_cost, best = cur, [list(a) for a in assign]
    return best


def _build_graph(slot_lens):
    """Build + compile the SPMD Bacc graph. slot_lens: [GPC][4] ints (1..256)."""
    import concourse.bass as bass
    import concourse.bacc as bacc
    import concourse.mybir as mybir
    import concourse.tile as tile
    from concourse.masks import make_identity

    f32 = mybir.dt.float32
    bf = mybir.dt.bfloat16
    f8 = mybir.dt.float8e4
    AX = mybir.AxisListType
    AF = mybir.ActivationFunctionType
    DR = mybir.MatmulPerfMode.DoubleRow

    nc = bacc.Bacc("TRN2", target_bir_lowering=False, debug=False,
                   num_devices=N_CORES)

    p_bf_d = nc.dram_tensor("p_bf", [GPC, HC, 128, 4, S], bf, kind="ExternalInput")
    mask_d = nc.dram_tensor("maskbc", [GPC, 128, 4, S + 1], bf, kind="ExternalInput")
    wt_d = nc.dram_tensor("wt", [128, HC, HC, 128], bf, kind="ExternalInput")
    wd_d = nc.dram_tensor("wd", [128, HC, H], bf, kind="ExternalInput")
    wd3_d = nc.dram_tensor("wd3", [3, 128, 3, HC, 2, 128], f8, kind="ExternalInput")
    w1_d = nc.dram_tensor("w1", [128, HC, HC, 128], bf, kind="ExternalInput")
    w2_d = nc.dram_tensor("w2", [128, HC, HC, 128], bf, kind="ExternalInput")
    bias_d = nc.dram_tensor("biases", [128, 4, HC], f32, kind="ExternalInput")
    out_d = nc.dram_tensor("out", [GPC, 4, HC, 128, S], f32, kind="ExternalOutput")

    with tile.TileContext(nc) as tc:
        with (
            tc.tile_pool(name="const", bufs=1) as constp,
            tc.tile_pool(name="wres", bufs=1) as wres,
            tc.tile_pool(name="wstream", bufs=2) as wstream,
            tc.tile_pool(name="gin", bufs=1) as gin,
            tc.tile_pool(name="act", bufs=1) as act,
            tc.tile_pool(name="smp", bufs=2) as smp,
            tc.tile_pool(name="fin", bufs=2) as fin,
            tc.tile_pool(name="patt", bufs=4, space="PSUM") as patt,
            tc.tile_pool(name="psbig", bufs=1, space="PSUM") as psbig,
            tc.tile_pool(name="pshalf", bufs=1, space="PSUM") as pshalf,
            tc.tile_pool(name="pstr", bufs=1, space="PSUM") as pstr,
        ):
            ident_f = constp.tile([128, 128], f32)
            make_identity(nc, ident_f[:])
            ident = constp.tile([128, 128], bf)
            nc.vector.tensor_copy(ident[:], ident_f[:])
            biases = constp.tile([128, 4, HC], f32)
            nc.sync.dma_start(biases[:], bias_d.ap())
            bt_b = biases[:, 0, :]
            bd_b = biases[:, 1, :]
            bd3_b = biases[:, 2, :]
            b12_b = biases[:, 3, :]

            wt_sb = wres.tile([128, HC, HC, 128], bf)
            nc.sync.dma_start(wt_sb[:], wt_d.ap())

            # first group's inputs land before the remaining weights so the
            # tensor engine can start early
            pbf_t, msk_t = [], []
            for g in range(GPC):
                pbf_t.append(gin.tile([128, HC, 4, S], bf, tag="pbf", bufs=2,
                                      name=f"pbf{g}"))
                msk_t.append(gin.tile([128, 4, S + 1], bf, tag="msk", bufs=2,
                                      name=f"msk{g}"))

            def load_group(g):
                for hc in range(HC):
                    nc.sync.dma_start(pbf_t[g][:, hc, :, :], p_bf_d.ap()[g][hc])
                nc.sync.dma_start(msk_t[g][:], mask_d.ap()[g])

            load_group(0)

            wd_sb = wres.tile([128, HC, H], bf)
            nc.sync.dma_start(wd_sb[:], wd_d.ap())
            w1_sb = wres.tile([128, HC, HC, 128], bf)
            nc.sync.dma_start(w1_sb[:], w1_d.ap())
            w2_sb = wres.tile([128, HC, HC, 128], bf)
            nc.sync.dma_start(w2_sb[:], w2_d.ap())

            for g in range(GPC):
                L = [int(x) for x in slot_lens[g]]
                tcs = [max(1, math.ceil(l / 128)) for l in L]

                pbf, mskb = pbf_t[g], msk_t[g]
                if g + 1 < GPC:
                    load_group(g + 1)

                # ---- trans_t (feature-major, bf16): ttT[h', t] = Wt @ pT + bt
                # one extra column per option (index L[o]) zeroed so the score
                # matmuls can produce the epsilon column via accumulation
                ttT = act.tile([128, HC, 4, S + 1], bf, tag="ttT", bufs=2)
                for m in range(HC):
                    for o in range(4):
                        ps = patt.tile([128, 257], f32, tag="patt")
                        for hc in range(HC):
                            nc.tensor.matmul(
                                ps[:, 0:L[o]],
                                wt_sb[:, hc, m, :],
                                pbf[:, hc, o, 0:L[o]],
                                start=(hc == 0), stop=(hc == HC - 1),
                            )
                        nc.scalar.activation(
                            ttT[:, m, o, 0:L[o]], ps[:, 0:L[o]],
                            AF.Identity, bias=bt_b[:, m:m + 1],
                        )
                for o in range(4):
                    nc.vector.memset(ttT[:, :, o, L[o]:L[o] + 1], 0.0)

                # ---- trans_d (natural [t, h], bf16): td = pT^T @ WdT
                # td psum deliberately avoids the psbig tag: sharing it with
                # the final stage's zps (bufs=1) stalled td(g+1) on the last
                # sigmoid of group g at every group boundary
                td = act.tile([128, 4, 2, H], bf, tag="td")
                for o in range(4):
                    for tcx in range(tcs[o]):
                        w = min(128, L[o] - tcx * 128)
                        psA = pshalf.tile([128, 512], f32, tag="pshalf",
                                          name="tdA")
                        psB = patt.tile([128, 257], f32, tag="patt", name="tdB")
                        for hc in range(HC):
                            lhs = pbf[:, hc, o, tcx * 128: tcx * 128 + w]
                            nc.tensor.matmul(psA[0:w, 0:512], lhs, wd_sb[:, hc, 0:512],
                                             start=(hc == 0), stop=(hc == HC - 1))
                            nc.tensor.matmul(psB[0:w, 0:256], lhs, wd_sb[:, hc, 512:768],
                                             start=(hc == 0), stop=(hc == HC - 1))
                        nc.scalar.activation(td[0:w, o, tcx, 0:512], psA[0:w, 0:512],
                                             AF.Copy)
                        nc.scalar.activation(td[0:w, o, tcx, 512:768], psB[0:w, 0:256],
                                             AF.Copy)

                # stream Wd3 for this group
                wd3_sb = []
                for k in range(3):
                    t = wstream.tile([128, 3, HC, 2, 128], f8, tag=f"wd3_{k}", bufs=1)
                    nc.sync.dma_start(t[:], wd3_d.ap()[k])
                    wd3_sb.append(t)

                ocT = act.tile([128, HC, 4, S], bf, tag="ocT")
                for half in range(2):
                    ock = act.tile([128, 3, HC, 2, S], f8, tag="ock", bufs=2)
                    for io in range(2):
                        i = half * 2 + io
                        jlist = [j for j in range(4) if j != i]

                        def emit_scores(j):
                            # scores + softmax chain for one partner; the
                            # vector/scalar chain overlaps the next partner's
                            # score matmuls (software pipelining)
                            lj = L[j]
                            smc = []
                            for m in range(2):
                                ps = patt.tile([128, 257], f32, tag="patt",
                                               name="ps")
                                for hc in range(HC):
                                    nc.tensor.matmul(
                                        ps[:, 0:lj + 1],
                                        pbf[:, hc, i, m * 128:(m + 1) * 128],
                                        ttT[:, hc, j, 0:lj + 1],
                                        start=(hc == 0), stop=(hc == HC - 1),
                                    )
                                nc.vector.tensor_add(
                                    ps[:, 0:lj + 1], ps[:, 0:lj + 1],
                                    mskb[:, j, 0:lj + 1])
                                stats = smp.tile([128, 16], f32, tag="stats",
                                                 bufs=8, name="stats")
                                nc.vector.tensor_reduce(
                                    stats[:, 0:1], ps[:, 0:lj], AX.X,
                                    mybir.AluOpType.max, negate=True)
                                # -M = min(0, -max)
                                nc.vector.tensor_scalar_min(stats[:, 0:1], stats[:, 0:1], 0.0)
                                e = smp.tile([128, 257], f32, tag="e", bufs=4,
                                             name="e")
                                nc.scalar.activation(
                                    e[:, 0:lj + 1], ps[:, 0:lj + 1],
                                    AF.Exp, bias=stats[:, 0:1], scale=1.0,
                                    accum_out=stats[:, 1:2])
                                nc.vector.reciprocal(stats[:, 2:3], stats[:, 1:2])
                                sm = smp.tile([128, 256], bf, tag="sm", bufs=6,
                                              name="sm")
                                nc.vector.tensor_scalar_mul(sm[:, 0:lj], e[:, 0:lj],
                                                            stats[:, 2:3])
                                smc.append(sm)
                            return smc

                        def emit_transpose(j, smc):
                            lj = L[j]
                            smT = smp.tile([128, 2, 256], bf, tag="smT", bufs=3,
                                           name="smT")
                            for tcx in range(tcs[j]):
                                w = min(128, lj - tcx * 128)
                                tp = pstr.tile([128, 256], bf, tag="pstr",
                                               name="tp")
                                for m in range(2):
                                    nc.tensor.matmul(
                                        tp[0:w, m * 128:(m + 1) * 128],
                                        smc[m][:, tcx * 128: tcx * 128 + w],
                                        ident[:], is_transpose=True,
                                        start=(m == 0), stop=(m == 1))
                                nc.scalar.activation(smT[0:w, tcx, :], tp[0:w, :],
                                                     AF.Copy)
                            return smT

                        def emit_av(j, jr, smT):
                            # av: avT[h', s] += td_j^T(blocks) @ smT
                            lj = L[j]
                            for hc in range(HC):
                                aps = patt.tile([128, 257], f32, tag="patt",
                                                name="aps")
                                for tcx in range(tcs[j]):
                                    w = min(128, lj - tcx * 128)
                                    nc.tensor.matmul(
                                        aps[:, 0:256],
                                        td[0:w, j, tcx, hc * 128:(hc + 1) * 128],
                                        smT[0:w, tcx, :],
                                        start=(tcx == 0), stop=(tcx == tcs[j] - 1))
                                nc.scalar.activation(
                                    ock[:, jr, hc, io, :], aps[:, 0:256],
                                    AF.Relu, bias=bd_b[:, hc:hc + 1], scale=SCK)

                        smc0 = emit_scores(jlist[0])
                        smc1 = emit_scores(jlist[1])
                        smT0 = emit_transpose(jlist[0], smc0)
                        emit_av(jlist[0], 0, smT0)
                        smc2 = emit_scores(jlist[2])
                        smT1 = emit_transpose(jlist[1], smc1)
                        emit_av(jlist[1], 1, smT1)
                        smT2 = emit_transpose(jlist[2], smc2)
                        emit_av(jlist[2], 2, smT2)

                    # ---- oc for this half (2 options): ocT = sum_k Wd3_k @ ock_k + bd3
                    for m in range(HC):
                        ops = pshalf.tile([128, 512], f32, tag="pshalf")
                        step = 0
                        for k in range(3):
                            for hp in range(3):
                                nc.tensor.matmul(
                                    ops[:, :], wd3_sb[k][:, hp, m, :, :],
                                    ock[:, k, 2 * hp:2 * hp + 2, :, :],
                                    start=(step == 0), stop=(step == 8),
                                    perf_mode=DR)
                                step += 1
                        for oo in range(2):
                            o = half * 2 + oo
                            nc.scalar.activation(
                                ocT[:, m, o, :], ops[:, oo * 256:(oo + 1) * 256],
                                AF.Identity, bias=bd3_b[:, m:m + 1],
                                scale=1.0 / (SCK * SCW))

                # ---- mid + final output, per m-chunk
                for m in range(HC):
                    # w2 @ p first: no dependency on the oc stage, so the PE
                    # rolls straight into the final stage while the last ocT
                    # activations drain
                    zps = psbig.tile([128, 1024], f32, tag="psbig")
                    for hc in range(HC):
                        nc.tensor.matmul(zps[:, 0:512], w2_sb[:, hc, m, :],
                                         pbf[:, hc, 0:2, :],
                                         start=(hc == 0), stop=False)
                        nc.tensor.matmul(zps[:, 512:1024], w2_sb[:, hc, m, :],
                                         pbf[:, hc, 2:4, :],
                                         start=(hc == 0), stop=False)
                    for hc in range(HC):
                        nc.tensor.matmul(zps[:, 0:512], w1_sb[:, hc, m, :],
                                         ocT[:, hc, 0:2, :],
                                         start=False, stop=(hc == HC - 1))
                        nc.tensor.matmul(zps[:, 512:1024], w1_sb[:, hc, m, :],
                                         ocT[:, hc, 2:4, :],
                                         start=False, stop=(hc == HC - 1))
                    mid = fin.tile([128, 1024], bf, tag="mid", bufs=2)
                    nc.scalar.activation(mid[:], zps[:], AF.Sigmoid,
                                         bias=b12_b[:, m:m + 1])
                    for o in range(4):
                        d = fin.tile([128, 256], f32, tag="fd", bufs=3)
                        nc.gpsimd.tensor_sub(d[:], pbf[:, m, o, :], ocT[:, m, o, :])
                        nc.vector.tensor_mul(d[:], d[:], mid[:, o * 256:(o + 1) * 256])
                        fo = fin.tile([128, 256], f32, tag="fout", bufs=4)
                        nc.vector.tensor_add(fo[:], d[:], ocT[:, m, o, :])
                        nc.sync.dma_start(out_d.ap()[g][o][m], fo[:])

    nc.compile()
    return nc


def _pack_weights(Wt, bt, Wd, bd, Wd3, bd3, W1, b1, W2, b2):
    def lhs_blocks(w):  # [H,H] -> [128, HC(k), HC(m), 128] of W.T
        return np.ascontiguousarray(
            w.T.reshape(HC, 128, HC, 128).transpose(1, 0, 2, 3))

    wt = lhs_blocks(np.asarray(Wt, np.float32)).astype(BF16)
    w1 = lhs_blocks(np.asarray(W1, np.float32)).astype(BF16)
    w2 = lhs_blocks(np.asarray(W2, np.float32)).astype(BF16)
    wd = np.ascontiguousarray(
        np.asarray(Wd, np.float32).T.reshape(HC, 128, H).transpose(1, 0, 2)).astype(BF16)

    def wd3_block(k):  # [128, 3(hp), HC(m), 2, 128] fp8, DoubleRow pairing
        blk = np.ascontiguousarray(
            np.asarray(Wd3, np.float32)[:, k * H:(k + 1) * H].T
            .reshape(HC, 128, HC, 128).transpose(1, 0, 2, 3))
        blk = blk.reshape(128, 3, 2, HC, 128).transpose(0, 1, 3, 2, 4)
        return _clip8(np.ascontiguousarray(blk), SCW)

    wd3 = np.stack([wd3_block(k) for k in range(3)])
    biases = np.stack([
        np.asarray(v, np.float32).reshape(HC, 128).T
        for v in (bt, np.asarray(bd, np.float32) * SCK, bd3,
                  np.asarray(b1, np.float32) + np.asarray(b2, np.float32))
    ], axis=1)  # [128, 4, HC]
    biases = np.ascontiguousarray(biases, np.float32)
    return wt, wd, wd3, w1, w2, biases


def kernel(**inputs):
    from concourse.bass_utils import run_bass_kernel_spmd

    p = np.asarray(inputs["p"], np.float32)
    option_len = np.asarray(inputs["option_len"]).astype(np.int64)
    lens = (option_len + 1).astype(np.int64)  # [B] key lengths
    glens = lens.reshape(B // 4, 4)

    slots = _assign_groups(glens)  # [GPC][8] group ids
    slot_lens = tuple(
        tuple(int(glens[slots[g]].max(axis=0)[o]) for o in range(4))
        for g in range(GPC))

    if slot_lens not in _GRAPH_CACHE:
        _GRAPH_CACHE[slot_lens] = _build_graph(slot_lens)
    nc = _GRAPH_CACHE[slot_lens]

    wt, wd, wd3, w1, w2, biases = _pack_weights(
        inputs["Wt"], inputs["bt"], inputs["Wd"], inputs["bd"],
        inputs["Wd3"], inputs["bd3"], inputs["W1"], inputs["b1"],
        inputs["W2"], inputs["b2"])

    in_maps = []
    core_groups = []  # [core][g] -> group id
    for c in range(N_CORES):
        gids = [slots[g][c] for g in range(GPC)]
        core_groups.append(gids)
        opts = np.concatenate([np.arange(4) + 4 * gid for gid in gids])
        pc = p[opts]  # [16, S, H]
        pT = pc.transpose(0, 2, 1).reshape(GPC, 4, HC, 128, S)
        pT = np.ascontiguousarray(pT.transpose(0, 2, 3, 1, 4))  # [g, hc, p, o, s]
        maskrow = np.zeros((GPC, 1, 4, S + 1), np.float32)
        for g in range(GPC):
            for o in range(4):
                lc = int(glens[gids[g]][o])
                sl = int(slot_lens[g][o])
                maskrow[g, 0, o, lc:sl] = -30000.0
                maskrow[g, 0, o, sl] = math.log(1e-13 * (S - lc))
        maskbc = np.broadcast_to(maskrow, (GPC, 128, 4, S + 1))
        in_maps.append({
            "p_bf": pT.astype(BF16),
            "maskbc": np.ascontiguousarray(maskbc).astype(BF16),
            "wt": wt, "wd": wd, "wd3": wd3, "w1": w1, "w2": w2,
            "biases": biases,
        })

    try:
        res = run_bass_kernel_spmd(nc, in_maps, list(range(N_CORES)))
    except Exception:
        # a previously wedged device surfaces on the first execute; the
        # runtime resets it, so a single retry suffices
        res = run_bass_kernel_spmd(nc, in_maps, list(range(N_CORES)))

    out = np.empty((B, S, H), np.float32)
    for c in range(N_CORES):
        oc = res.results[c]["out"]  # [GPC, 4, HC, 128, S]
        for g in range(GPC):
            gid = core_groups[c][g]
            # [4, HC, 128, S] -> [4, S, H]
            blk = oc[g].transpose(0, 3, 1, 2).reshape(4, S, H)
            out[4 * gid: 4 * gid + 4] = blk
    return out
